# revision 1
# baseline (speedup 1.0000x reference)
"""EnergyGCN Trainium2 kernel: 8-core SPMD Bass/Tile implementation.

Strategy (node sharding, per the sharding hint):
  - 50000 nodes sharded contiguously across 8 cores (6250 dest nodes/core).
  - Per layer: each core computes hr = h @ Wr[l].T for its shard, scales rows
    by dinv (h~ = dinv * hr), AllGathers h~ into a full [50000,128] HBM table.
  - Edges are dest-sorted on the host, grouped into 128-edge chunks per
    128-dest block.  Messages h~[col] are fetched with dma_gather (SWDGE) and
    aggregated on the TensorEngine with one-hot "segment matrices" S generated
    on-chip by DVE iota/is_equal; PSUM accumulates
        psum[d,:] = sum_e 1[dest_e==d] * h~[col_e]        (chunks)
                  + (-2*deg_d) * h~[d]                    (identity inject 1)
                  + gate_l * (-1/(3*dinv_d)) * h0[d]      (identity inject 2)
    and the epilogue h0_new = (-3*dinv_d) * psum gives exactly
        h0_new = gate_l*h0 + 6*hr - 3*ahat(hr).
  - relu is applied on read (next layer / final lin2).

Reference math:
    h = relu(x @ W1 + b1); h0 = h
    for l: hr = h @ Wr[l].T ; hn = 6hr - 3*ahat(hr)
           h0 = (1+tanh(eps[l]))*h0 + hn ; h = relu(h0)
    out = h @ W2 + b2
with ahat(y) = segment_sum(w[:,None]*y[col], row), w = dinv[row]*dinv[col],
self-loops appended, deg = counts of row, dinv = rsqrt(deg).
"""

import math

import numpy as np

import concourse.bacc as bacc
import concourse.bass as bass
import concourse.mybir as mybir
import concourse.tile as tile
from concourse import bass_utils, library_config

F32 = mybir.dt.float32
AF = mybir.ActivationFunctionType
ALU = mybir.AluOpType

N_NODES = 50000
D_IN, D_H, D_OUT = 256, 128, 64
N_LAYERS = 4
N_CORES = 8


class Cfg:
    def __init__(self, n=N_NODES, n_cores=N_CORES, d_in=D_IN, d_h=D_H,
                 d_out=D_OUT, n_layers=N_LAYERS, sg_blocks=4, sgen_batch=8,
                 gather_bufs=2, split_at=25000, max_call_chunks=16,
                 single_packet=False):
        self.n = n
        self.n_cores = n_cores
        self.d_in = d_in
        self.d_h = d_h
        self.d_out = d_out
        self.n_layers = n_layers
        self.sg_blocks = sg_blocks
        self.sgen_batch = sgen_batch
        self.gather_bufs = gather_bufs
        self.split_at = split_at
        self.max_call_chunks = max_call_chunks
        self.single_packet = single_packet
        assert n % n_cores == 0
        self.nsh = n // n_cores
        self.nb = (self.nsh + 127) // 128


class Plan:
    pass


def preprocess(edge_index: np.ndarray, cfg: Cfg) -> Plan:
    n, P, nsh, nb = cfg.n, cfg.n_cores, cfg.nsh, cfg.nb
    row = np.concatenate([edge_index[0], np.arange(n, dtype=np.int64)])
    col = np.concatenate([edge_index[1], np.arange(n, dtype=np.int64)])
    deg = np.bincount(row, minlength=n).astype(np.float64)
    dinv = np.where(deg > 0, 1.0 / np.sqrt(deg), 0.0)

    core = row // nsh
    per_core_edges = []
    counts = np.zeros((P, nb, 2), dtype=np.int64)
    for r in range(P):
        m = core == r
        rr = (row[m] - r * nsh).astype(np.int64)
        cc = col[m].astype(np.int64)
        g = np.zeros_like(cc)
        blk = rr // 128
        order = np.lexsort((rr, g, blk))
        rr, cc, g, blk = rr[order], cc[order], g[order], blk[order]
        per_core_edges.append((rr, cc, g, blk))
        np.add.at(counts[r], (blk, g), 1)

    nchunks = (counts.max(axis=0) + 127) // 128            # [nb, 2]

    # chunk layout: supergroups of sg_blocks blocks; per sg: (grp0 chunks for
    # its blocks in order) then (grp1 chunks).  One dma_gather per (sg, grp).
    slot_off = {}
    sg_entries = []
    off = 0
    n_sg = (nb + cfg.sg_blocks - 1) // cfg.sg_blocks
    total_chunks = 0
    for s in range(n_sg):
        blocks = list(range(s * cfg.sg_blocks, min((s + 1) * cfg.sg_blocks, nb)))
        entries = []
        for g in (0, 1):
            c0 = off
            for b in blocks:
                slot_off[(b, g)] = off
                off += int(nchunks[b, g]) * 128
            entries.append((g, (off - c0) // 128, c0))
        sg_entries.append((blocks, entries))
    total_slots = off
    total_chunks = total_slots // 128

    per_core = []
    for r in range(P):
        rr, cc, g, blk = per_core_edges[r]
        idx = np.zeros(total_slots, dtype=np.int64)
        dstrel = np.full(total_slots, -1.0, dtype=np.float32)
        for b in range(nb):
            for gg in (0, 1):
                m = (blk == b) & (g == gg)
                k = int(m.sum())
                if k == 0:
                    continue
                o = slot_off[(b, gg)]
                idx[o:o + k] = cc[m]
                dstrel[o:o + k] = (rr[m] - b * 128).astype(np.float32)
        idx128 = np.ascontiguousarray(
            idx.astype(np.int32).reshape(total_chunks, 128).T)  # [128, nchunk]
        dstrel128 = np.ascontiguousarray(
            dstrel.reshape(total_chunks, 128).T)            # [128, nchunk]

        dloc = dinv[r * nsh:(r + 1) * nsh]
        degloc = deg[r * nsh:(r + 1) * nsh]

        def colmat(v):
            out = np.zeros((nb * 128,), dtype=np.float64)
            out[:nsh] = v
            return np.ascontiguousarray(out.reshape(nb, 128).T.astype(np.float32))

        per_core.append(dict(
            idx_tbl=idx128,
            dstrel=dstrel128,
            dinv_cols=colmat(dloc),
            s_cols=colmat(-2.0 * degloc),
            s2_cols=colmat(np.where(dloc > 0, -1.0 / (3.0 * dloc), 0.0)),
            m3dinv_cols=colmat(-3.0 * dloc),
        ))

    plan = Plan()
    plan.cfg = cfg
    plan.nchunks = nchunks
    plan.sg_entries = sg_entries
    plan.total_chunks = total_chunks
    plan.total_slots = total_slots
    plan.per_core = per_core
    return plan


def build_bass(plan: Plan, gates, debug_dump=False):
    cfg = plan.cfg
    nsh, nb, P = cfg.nsh, cfg.nb, cfg.n_cores
    H, DI, DO, L = cfg.d_h, cfg.d_in, cfg.d_out, cfg.n_layers
    total_chunks = plan.total_chunks
    total_slots = plan.total_slots
    lo_rows = min(cfg.split_at, cfg.n)

    nc = bacc.Bacc("TRN2", target_bir_lowering=False, debug=False,
                   num_devices=P)

    KI = DI // 128
    xT = nc.dram_tensor("xT", [128, KI * nsh], F32, kind="ExternalInput")
    W1 = nc.dram_tensor("W1", [128, KI * H], F32, kind="ExternalInput")
    b1r = nc.dram_tensor("b1r", [1, H], F32, kind="ExternalInput")
    WrT = nc.dram_tensor("WrT", [128, L * H], F32, kind="ExternalInput")
    W2 = nc.dram_tensor("W2", [H, DO], F32, kind="ExternalInput")
    b2r = nc.dram_tensor("b2r", [1, DO], F32, kind="ExternalInput")
    idx_tbl_d = nc.dram_tensor("idx_tbl", [128, total_chunks],
                               mybir.dt.int32, kind="ExternalInput")
    iota_d = nc.dram_tensor("iota_in", [128, 8 * 128], F32, kind="ExternalInput")
    ident_d = nc.dram_tensor("ident_in", [128, 128], F32, kind="ExternalInput")
    dstrel_d = nc.dram_tensor("dstrel", [128, total_chunks], F32,
                              kind="ExternalInput")
    dinv_d = nc.dram_tensor("dinv_cols", [128, nb], F32, kind="ExternalInput")
    s_d = nc.dram_tensor("s_cols", [128, nb], F32, kind="ExternalInput")
    s2_d = nc.dram_tensor("s2_cols", [128, nb], F32, kind="ExternalInput")
    m3_d = nc.dram_tensor("m3dinv_cols", [128, nb], F32, kind="ExternalInput")
    out_d = nc.dram_tensor("out", [nsh, DO], F32, kind="ExternalOutput")
    if debug_dump:
        dbg_htl = nc.dram_tensor("dbg_htl", [nsh, H], F32, kind="ExternalOutput")
        dbg_tbl = nc.dram_tensor("dbg_tbl", [cfg.n, H], F32, kind="ExternalOutput")
        dbg_h0 = nc.dram_tensor("dbg_h0", [nsh, H], F32, kind="ExternalOutput")
        dbg_mid = nc.dram_tensor("dbg_mid", [128, H], F32, kind="ExternalOutput")
        dbg_i2 = nc.dram_tensor("dbg_i2", [128, 128], F32, kind="ExternalOutput")
        dbg_is = nc.dram_tensor("dbg_is", [128, 128], F32, kind="ExternalOutput")

    last_rows = nsh - (nb - 1) * 128

    with tile.TileContext(nc) as tc:
        with (
            tc.tile_pool(name="const", bufs=1) as cpool,
            tc.tile_pool(name="work", bufs=3) as work,
            tc.tile_pool(name="sgen", bufs=3) as sgen_pool,
            tc.tile_pool(name="gbuf", bufs=cfg.gather_bufs) as gpool,
            tc.tile_pool(name="pt", bufs=2, space="PSUM") as pt_pool,
            tc.tile_pool(name="ph", bufs=2, space="PSUM") as ph_pool,
            tc.tile_pool(name="pagg", bufs=2, space="PSUM") as pagg_pool,
            tc.tile_pool(name="dram", bufs=2, space="DRAM") as dram,
        ):
            # persistent per-block state
            h0_t = [cpool.tile([128, 128], F32, tag=f"h0_{b}", name=f"h0_{b}") for b in range(nb)]
            htl_t = [cpool.tile([128, H], F32, tag=f"htl_{b}", name=f"htl_{b}") for b in range(nb)]
            for b in range(nb):
                nc.vector.memset(h0_t[b][:], 0.0)
                nc.vector.memset(htl_t[b][:], 0.0)

            idx_sb = cpool.tile([128, total_chunks], mybir.dt.int32)
            nc.sync.dma_start(idx_sb[:], idx_tbl_d[:, :])
            dstrel_sb = cpool.tile([128, total_chunks], F32)
            nc.sync.dma_start(dstrel_sb[:], dstrel_d[:, :])
            dinv_sb = cpool.tile([128, nb], F32)
            nc.sync.dma_start(dinv_sb[:], dinv_d[:, :])
            s_sb = cpool.tile([128, nb], F32)
            nc.sync.dma_start(s_sb[:], s_d[:, :])
            s2_sb = cpool.tile([128, nb], F32)
            nc.sync.dma_start(s2_sb[:], s2_d[:, :])
            m3_sb = cpool.tile([128, nb], F32)
            nc.sync.dma_start(m3_sb[:], m3_d[:, :])

            W1_sb = cpool.tile([128, KI * H], F32)
            nc.sync.dma_start(W1_sb[:], W1[:, :])
            b1_sb = cpool.tile([1, H], F32)
            nc.sync.dma_start(b1_sb[:], b1r[:, :])
            WrT_sb = cpool.tile([128, L * H], F32)
            nc.sync.dma_start(WrT_sb[:], WrT[:, :])
            W2_sb = cpool.tile([H, DO], F32)
            nc.sync.dma_start(W2_sb[:], W2[:, :])
            b2_sb = cpool.tile([1, DO], F32)
            nc.sync.dma_start(b2_sb[:], b2r[:, :])
            ones_sb = cpool.tile([1, 128], F32)
            nc.vector.memset(ones_sb[:], 1.0)

            iota_sb = cpool.tile([128, cfg.sgen_batch * 128], F32)
            nc.sync.dma_start(iota_sb[:], iota_d[:, :cfg.sgen_batch * 128])
            ident = cpool.tile([128, 128], F32)
            nc.sync.dma_start(ident[:], ident_d[:, :])
            Is_t = [cpool.tile([128, 128], F32, tag=f"is_{b}", name=f"is_{b}") for b in range(nb)]
            for b in range(nb):
                nc.vector.tensor_scalar(Is_t[b][:], ident[:],
                                        s_sb[:, b:b + 1], None, op0=ALU.mult)

            # ---- lin1: h0 = x @ W1 + b1 ----
            for b in range(nb):
                rows = last_rows if b == nb - 1 else 128
                ps = ph_pool.tile([128, H], F32, tag="ph")
                for k in range(KI):
                    xs = work.tile([128, 128], F32, tag="xs")
                    nc.sync.dma_start(
                        xs[:, :rows],
                        xT[:, k * nsh + b * 128:k * nsh + b * 128 + rows])
                    nc.tensor.matmul(ps[:rows, :], xs[:, :rows],
                                     W1_sb[:, k * H:(k + 1) * H],
                                     start=(k == 0), stop=False)
                nc.tensor.matmul(ps[:rows, :], ones_sb[:, :rows], b1_sb[:],
                                 start=False, stop=True)
                # reference: h0 = relu(x @ W1 + b1) -- the initial residual
                # state is the post-relu activation
                nc.scalar.activation(h0_t[b][:rows, :], ps[:rows, :], AF.Relu)

            # ---- layers ----
            for l in range(L):
                bounce = dram.tile([nsh, H], F32, tag="bounce")
                table = dram.tile([cfg.n, H], F32, tag="table")
                for b in range(nb):
                    rows = last_rows if b == nb - 1 else 128
                    hb = work.tile([128, 128], F32, tag="hrelu")
                    nc.scalar.activation(hb[:], h0_t[b][:], AF.Relu)
                    ptp = pt_pool.tile([128, 128], F32, tag="pt")
                    nc.tensor.transpose(ptp[:], hb[:], ident[:])
                    hT = work.tile([128, 128], F32, tag="hT")
                    nc.scalar.activation(hT[:], ptp[:], AF.Copy)
                    php = ph_pool.tile([128, H], F32, tag="ph")
                    nc.tensor.matmul(php[:rows, :], hT[:, :rows],
                                     WrT_sb[:, l * H:(l + 1) * H],
                                     start=True, stop=True)
                    nc.scalar.activation(htl_t[b][:rows, :], php[:rows, :],
                                         AF.Copy, scale=dinv_sb[:rows, b:b + 1])
                    nc.sync.dma_start(bounce[b * 128:b * 128 + rows, :],
                                      htl_t[b][:rows, :])
                nc.gpsimd.collective_compute(
                    "AllGather", ALU.bypass,
                    replica_groups=[list(range(P))],
                    ins=[bounce.opt()], outs=[table.opt()],
                )
                if debug_dump and l == 0:
                    nc.sync.dma_start(dbg_htl[:, :], bounce.opt())
                    nc.sync.dma_start(dbg_tbl[:, :], table.opt())

                tbl = table.opt()
                for (blocks, entries) in plan.sg_entries:
                    bufs = {}
                    base_chunk = {}
                    for (g, nch, c0) in entries:
                        if nch == 0:
                            continue
                        mb = gpool.tile([128, nch, H], F32, tag=f"g{g}")
                        ch0 = c0 // 128
                        for q0 in range(nch):
                            nc.gpsimd.indirect_dma_start(
                                mb[:, q0, :], None,
                                tbl,
                                bass.IndirectOffsetOnAxis(
                                    ap=idx_sb[:, ch0 + q0:ch0 + q0 + 1],
                                    axis=0))
                        bufs[g] = mb
                        base_chunk[g] = ch0
                    for b in blocks:
                        rows = last_rows if b == nb - 1 else 128
                        nch_tot = int(plan.nchunks[b, 0] + plan.nchunks[b, 1])
                        pa = pagg_pool.tile([128, H], F32, tag="pagg")
                        nc.tensor.matmul(pa[:], Is_t[b][:], htl_t[b][:],
                                         start=True, stop=False)
                        i2 = work.tile([128, 128], F32, tag="i2")
                        nc.vector.tensor_scalar(i2[:], ident[:],
                                                s2_sb[:, b:b + 1], gates[l],
                                                op0=ALU.mult, op1=ALU.mult)
                        nc.tensor.matmul(pa[:], i2[:], h0_t[b][:],
                                         start=False, stop=(nch_tot == 0))
                        if debug_dump and l == 0 and b == 0:
                            mid = work.tile([128, H], F32, tag="dbgmid")
                            nc.scalar.activation(mid[:], pa[:], AF.Copy)
                            nc.sync.dma_start(dbg_mid[:, :], mid[:])
                            i2c = work.tile([128, 128], F32, tag="dbgi2")
                            nc.vector.tensor_copy(i2c[:], i2[:])
                            nc.sync.dma_start(dbg_i2[:, :], i2c[:])
                            isc = work.tile([128, 128], F32, tag="dbgis")
                            nc.vector.tensor_copy(isc[:], Is_t[b][:])
                            nc.sync.dma_start(dbg_is[:, :], isc[:])
                        done = 0
                        for g in (0, 1):
                            nch_bg = int(plan.nchunks[b, g])
                            if nch_bg == 0:
                                continue
                            loc = sum(int(plan.nchunks[bb, g])
                                      for bb in blocks if bb < b)
                            flat0 = base_chunk[g] + loc
                            mb = bufs[g]
                            for t0 in range(0, nch_bg, cfg.sgen_batch):
                                tn = min(cfg.sgen_batch, nch_bg - t0)
                                sg_t = sgen_pool.tile(
                                    [128, cfg.sgen_batch * 128], F32, tag="sg")
                                dsl = dstrel_sb[:, flat0 + t0:flat0 + t0 + tn]
                                nc.vector.tensor_tensor(
                                    sg_t[:, :tn * 128].rearrange(
                                        "p (c d) -> p c d", d=128),
                                    dsl.unsqueeze(2).broadcast_to([128, tn, 128]),
                                    iota_sb[:, :tn * 128].rearrange(
                                        "p (c d) -> p c d", d=128),
                                    op=ALU.is_equal)
                                for t in range(tn):
                                    done += 1
                                    nc.tensor.matmul(
                                        pa[:],
                                        sg_t[:, t * 128:(t + 1) * 128],
                                        mb[:, loc + t0 + t, :],
                                        start=False, stop=(done == nch_tot))
                        nc.scalar.activation(h0_t[b][:rows, :], pa[:rows, :],
                                             AF.Copy,
                                             scale=m3_sb[:rows, b:b + 1])

            if debug_dump:
                for b in range(nb):
                    rows = last_rows if b == nb - 1 else 128
                    nc.sync.dma_start(dbg_h0[b * 128:b * 128 + rows, :],
                                      h0_t[b][:rows, :])
            # ---- lin2 ----
            for b in range(nb):
                rows = last_rows if b == nb - 1 else 128
                hb = work.tile([128, 128], F32, tag="hrelu")
                nc.scalar.activation(hb[:], h0_t[b][:], AF.Relu)
                ptp = pt_pool.tile([128, 128], F32, tag="pt")
                nc.tensor.transpose(ptp[:], hb[:], ident[:])
                hT = work.tile([128, 128], F32, tag="hT")
                nc.scalar.activation(hT[:], ptp[:], AF.Copy)
                po = ph_pool.tile([128, DO], F32, tag="po")
                nc.tensor.matmul(po[:rows, :], hT[:, :rows], W2_sb[:, :],
                                 start=True, stop=False)
                nc.tensor.matmul(po[:rows, :], ones_sb[:, :rows], b2_sb[:],
                                 start=False, stop=True)
                ot = work.tile([128, DO], F32, tag="ot")
                nc.scalar.activation(ot[:rows, :], po[:rows, :], AF.Copy)
                nc.sync.dma_start(out_d[b * 128:b * 128 + rows, :], ot[:rows, :])

    nc.finalize()
    return nc


def make_in_maps(plan: Plan, x, W1, b1, Wr, W2, b2):
    cfg = plan.cfg
    nsh = cfg.nsh
    KI = cfg.d_in // 128
    W1 = np.ascontiguousarray(
        np.asarray(W1, np.float32).reshape(KI, 128, cfg.d_h)
        .transpose(1, 0, 2).reshape(128, KI * cfg.d_h))
    # WrT[k, l*H+j] = Wr[l, j, k]
    WrT = np.ascontiguousarray(
        np.asarray(Wr, np.float32).transpose(2, 0, 1).reshape(128, -1))
    iota_in = np.ascontiguousarray(
        np.tile(np.arange(128, dtype=np.float32), 8)[None, :].repeat(128, 0))
    ident_in = np.eye(128, dtype=np.float32)
    common = dict(
        W1=W1, b1r=np.ascontiguousarray(np.asarray(b1, np.float32).reshape(1, -1)),
        WrT=WrT, W2=np.ascontiguousarray(np.asarray(W2, np.float32)),
        b2r=np.ascontiguousarray(np.asarray(b2, np.float32).reshape(1, -1)),
        iota_in=iota_in, ident_in=ident_in,
    )
    in_maps = []
    for r in range(cfg.n_cores):
        pc = plan.per_core[r]
        # xT[p, k*nsh + c] = x[c, k*128 + p]
        xT = np.ascontiguousarray(
            np.asarray(x[r * nsh:(r + 1) * nsh], np.float32).T
            .reshape(KI, 128, nsh).transpose(1, 0, 2).reshape(128, KI * nsh))
        m = dict(common)
        m.update(
            xT=xT, idx_tbl=pc["idx_tbl"], dstrel=pc["dstrel"],
            dinv_cols=pc["dinv_cols"], s_cols=pc["s_cols"],
            s2_cols=pc["s2_cols"], m3dinv_cols=pc["m3dinv_cols"],
        )
        in_maps.append(m)
    return in_maps


_cache = {}


def kernel(x, W1, b1, Wr, eps, W2, b2, edge_index, *, trace=False, cfg=None):
    cfg = cfg or Cfg()
    x = np.asarray(x)
    edge_index = np.asarray(edge_index)
    gates = [float(1.0 + math.tanh(float(e))) for e in np.asarray(eps)]

    ck = hash((edge_index.tobytes(), tuple(gates), cfg.n, cfg.n_cores, cfg.split_at))
    if ck not in _cache:
        plan = preprocess(edge_index, cfg)
        nc = build_bass(plan, gates)
        _cache.clear()
        _cache[ck] = (plan, nc)
    plan, nc = _cache[ck]

    in_maps = make_in_maps(plan, x, W1, b1, Wr, W2, b2)
    try:
        res = bass_utils.run_bass_kernel_spmd(
            nc, in_maps, core_ids=list(range(cfg.n_cores)), trace=trace)
    except ModuleNotFoundError:
        # axon NTFF profiling hook unavailable in this container
        res = bass_utils.run_bass_kernel_spmd(
            nc, in_maps, core_ids=list(range(cfg.n_cores)), trace=False)
    out = np.concatenate([r["out"] for r in res.results], axis=0)
    kernel.last_results = res
    return out.astype(np.float32)



# revision 10
# speedup vs baseline: 1.3610x; 1.3610x over previous
"""EnergyGCN Trainium2 kernel: 8-core SPMD Bass/Tile implementation.

Strategy (node sharding):
  - 50000 nodes sharded contiguously across 8 cores (6250 rows, 49 blocks of
    128 per core).  Hidden state h0 kept TRANSPOSED in SBUF: h0T[b] =
    [H=128 partitions, 128 nodes] so the per-layer lin_right matmul needs no
    transposes (lhsT = WrT, rhs = relu(h0T)).
  - Per layer l: hrT = Wr[l] @ relu(h0T) (PSUM), cast fp16, PE-transpose to
    node-major, scaled by dinv on the PSUM->SBUF copy: hsb[b] = dinv * hr
    (= h~, fp16, node-major).  hsb is DMA'd into one of two DRAM bounce
    buffers (blocks 25-48 -> half A first, then 0-24 -> half B) and each half
    is AllGather'd (Shared output) into a [8*half, H] fp16 table.
  - Edge messages h~[col] are fetched with batched gpsimd.dma_gather (int16
    row indices into the half tables, ~6 dest blocks of edges per call) and
    aggregated on TensorE with one-hot segment matrices generated by DVE:
        S'[e, d] = (iota_d == dstrel_e) * (-3*dinv_dest_e)
    so PSUM accumulates the already-scaled -3*ahat edge sum:
        paT[h, d] = sum_e S'[e,d] * msg[e,h]  (+ C2 identity inject)
    with C2 = diag(6/dinv_d - 3*dinv_d) injecting (6 - 3*dinv^2)*hr = the
    6*hr term plus the self-loop correction.  Epilogue (DVE):
        h0T = gate_l * h0T + paT.
  - relu applied on read (next layer / final lin2).  lin1/lin2 are plain
    fp16 matmuls on the transposed state (no transposes needed).

Reference math:
    h = relu(x @ W1 + b1); h0 = h
    for l: hr = h @ Wr[l].T ; hn = 6hr - 3*ahat(hr)
           h0 = (1+tanh(eps[l]))*h0 + hn ; h = relu(h0)
    out = h @ W2 + b2
with ahat(y) = segment_sum(w[:,None]*y[col], row, n), w = dinv[row]*dinv[col],
self-loops appended, deg = counts of row (incl self), dinv = rsqrt(deg).
"""

import math

import numpy as np

import concourse.bacc as bacc
import concourse.bass as bass
import concourse.mybir as mybir
import concourse.tile as tile
from concourse import bass_utils

F32 = mybir.dt.float32
F16 = mybir.dt.float16
I16 = mybir.dt.int16
AF = mybir.ActivationFunctionType
ALU = mybir.AluOpType

N_NODES = 50000
D_IN, D_H, D_OUT = 256, 128, 64
N_LAYERS = 4
N_CORES = 8


class Cfg:
    def __init__(self, n=N_NODES, n_cores=N_CORES, d_in=D_IN, d_h=D_H,
                 d_out=D_OUT, n_layers=N_LAYERS, sg_blocks=6, gather_bufs=2,
                 use_dma_gather=True):
        self.n = n
        self.n_cores = n_cores
        self.d_in = d_in
        self.d_h = d_h
        self.d_out = d_out
        self.n_layers = n_layers
        self.sg_blocks = sg_blocks
        self.gather_bufs = gather_bufs
        self.use_dma_gather = use_dma_gather
        assert n % n_cores == 0
        self.nsh = n // n_cores
        self.nb = (self.nsh + 127) // 128
        # half B = blocks [0, nbB), allgathered second; half A first
        self.nbB = (self.nb + 1) // 2
        self.half_b = 128 * self.nbB
        self.ord_blocks = list(range(self.nbB, self.nb)) + list(range(self.nbB))


class Plan:
    pass


def preprocess(edge_index: np.ndarray, cfg: Cfg) -> Plan:
    n, P, nsh, nb = cfg.n, cfg.n_cores, cfg.nsh, cfg.nb
    row = edge_index[0].astype(np.int64)
    col = edge_index[1].astype(np.int64)
    deg = np.bincount(row, minlength=n).astype(np.float64) + 1.0  # + self loop
    dinv = 1.0 / np.sqrt(deg)
    m3 = (-3.0 * dinv).astype(np.float32)          # -3*dinv[dest]
    c2 = (6.0 / dinv - 3.0 * dinv).astype(np.float32)

    # source node -> (group, row in half table);  g=0: half A (off>=half_b)
    HB = cfg.half_b
    s = col // nsh
    off = col - s * nsh
    hA = nsh - HB
    g_of = (off < HB).astype(np.int64)              # g=0 -> half A, g=1 -> B
    rowg = np.where(off >= HB, s * hA + off - HB, s * HB + off)

    ordpos = np.zeros(nb, dtype=np.int64)           # block -> position
    for i, b in enumerate(cfg.ord_blocks):
        ordpos[b] = i
    SGB = cfg.sg_blocks

    core = row // nsh
    per_core_edges = []
    counts = np.zeros((P, nb, 2), dtype=np.int64)
    for r in range(P):
        m = core == r
        rr = row[m] - r * nsh
        cc = rowg[m]
        g = g_of[m]
        m3e = m3[row[m]]
        blk = rr // 128
        pos = ordpos[blk]
        order = np.lexsort((rr, pos, g, pos // SGB))
        per_core_edges.append((rr[order], cc[order], g[order], blk[order],
                               m3e[order]))
        np.add.at(counts[r], (blk, g), 1)

    nchunks = (counts.max(axis=0) + 127) // 128     # [nb, 2]

    # chunk layout: supergroups of SGB blocks in ord_blocks order; per sg:
    # (g0 chunks for its blocks in order) then (g1 chunks).
    slot_off = {}
    sg_entries = []
    offc = 0
    for i0 in range(0, nb, SGB):
        blocks = cfg.ord_blocks[i0:i0 + SGB]
        entries = []
        for g in (0, 1):
            c0 = offc
            for b in blocks:
                slot_off[(b, g)] = offc
                offc += int(nchunks[b, g]) * 128
            entries.append((g, (offc - c0) // 128, c0 // 128))
        sg_entries.append((blocks, entries))
    total_slots = offc
    total_chunks = total_slots // 128

    per_core = []
    for r in range(P):
        rr, cc, g, blk, m3e = per_core_edges[r]
        idx = np.zeros(total_slots, dtype=np.int16)     # pad -> row 0 (valid)
        dstrel = np.full(total_slots, -1.0, dtype=np.float32)
        m3slot = np.zeros(total_slots, dtype=np.float32)
        for b in range(nb):
            for gg in (0, 1):
                msel = (blk == b) & (g == gg)
                k = int(msel.sum())
                if k == 0:
                    continue
                o = slot_off[(b, gg)]
                idx[o:o + k] = cc[msel]
                dstrel[o:o + k] = (rr[msel] - b * 128).astype(np.float32)
                m3slot[o:o + k] = m3e[msel]
        # dma_gather index wrapping: slot i -> [i % 16, i // 16]; the Q7
        # tx/rx cores read different partition groups -> replicate to 128.
        idx16 = np.ascontiguousarray(np.tile(idx.reshape(-1, 16).T, (8, 1)))
        idx32 = np.ascontiguousarray(
            idx.astype(np.int32).reshape(total_chunks, 128).T)
        dstrel128 = np.ascontiguousarray(
            dstrel.reshape(total_chunks, 128).T)        # [128, nchunk]
        m3e128 = np.ascontiguousarray(
            m3slot.reshape(total_chunks, 128).T)

        dloc = dinv[r * nsh:(r + 1) * nsh]
        c2loc = c2[r * nsh:(r + 1) * nsh]

        def colmat(v):
            out = np.zeros((nb * 128,), dtype=np.float64)
            out[:nsh] = v
            return np.ascontiguousarray(
                out.reshape(nb, 128).T.astype(np.float32))

        per_core.append(dict(
            idx16=idx16, idx32=idx32,
            dstrel=dstrel128,
            m3e=m3e128,
            dinv_cols=colmat(dloc),
            c2_cols=colmat(c2loc),
        ))

    plan = Plan()
    plan.cfg = cfg
    plan.nchunks = nchunks
    plan.sg_entries = sg_entries
    plan.total_chunks = total_chunks
    plan.total_slots = total_slots
    plan.per_core = per_core
    return plan


def build_bass(plan: Plan, gates):
    cfg = plan.cfg
    nsh, nb, P = cfg.nsh, cfg.nb, cfg.n_cores
    H, DI, DO, L = cfg.d_h, cfg.d_in, cfg.d_out, cfg.n_layers
    TC = plan.total_chunks
    TS16 = plan.total_slots // 16
    hA = nsh - cfg.half_b               # 3050
    rowsA, rowsB = P * hA, P * cfg.half_b   # table heights

    nc = bacc.Bacc("TRN2", target_bir_lowering=False, debug=False,
                   num_devices=P)

    KI = DI // 128
    xT = nc.dram_tensor("xT", [128, KI * nsh], F16, kind="ExternalInput")
    W1 = nc.dram_tensor("W1", [128, KI * H], F16, kind="ExternalInput")
    b1c = nc.dram_tensor("b1c", [128, 1], F32, kind="ExternalInput")
    WrT = nc.dram_tensor("WrT", [128, L * H], F16, kind="ExternalInput")
    W2 = nc.dram_tensor("W2", [H, DO], F16, kind="ExternalInput")
    b2r = nc.dram_tensor("b2r", [1, DO], F16, kind="ExternalInput")
    idx16_d = nc.dram_tensor("idx16", [128, TS16], I16, kind="ExternalInput")
    idx32_d = nc.dram_tensor("idx32", [128, TC], mybir.dt.int32,
                             kind="ExternalInput")
    dstrel_d = nc.dram_tensor("dstrel", [128, TC], F32, kind="ExternalInput")
    m3e_d = nc.dram_tensor("m3e", [128, TC], F32, kind="ExternalInput")
    dinv_d = nc.dram_tensor("dinv_cols", [128, nb], F32, kind="ExternalInput")
    c2_d = nc.dram_tensor("c2_cols", [128, nb], F32, kind="ExternalInput")
    iota_d = nc.dram_tensor("iota_in", [128, 128], F16, kind="ExternalInput")
    ident_d = nc.dram_tensor("ident_in", [128, 128], F16, kind="ExternalInput")
    out_d = nc.dram_tensor("out", [nsh, DO], F32, kind="ExternalOutput")

    last_rows = nsh - (nb - 1) * 128

    def rows_of(b):
        return last_rows if b == nb - 1 else 128

    with tile.TileContext(nc) as tc:
        with (
            tc.tile_pool(name="const", bufs=1) as cpool,
            tc.tile_pool(name="work", bufs=4) as work,
            tc.tile_pool(name="sgen", bufs=6) as sgen_pool,
            tc.tile_pool(name="ga", bufs=cfg.gather_bufs) as ga_pool,
            tc.tile_pool(name="gb", bufs=cfg.gather_bufs) as gb_pool,
            tc.tile_pool(name="pt", bufs=2, space="PSUM") as pt_pool,
            tc.tile_pool(name="ph", bufs=2, space="PSUM") as ph_pool,
            tc.tile_pool(name="pagg", bufs=3, space="PSUM") as pagg_pool,
            tc.tile_pool(name="dram", bufs=2, space="DRAM") as dram,
        ):
            # persistent per-block state
            h0T = [cpool.tile([128, 128], F32, tag=f"h0_{b}", name=f"h0_{b}")
                   for b in range(nb)]
            hsb = [cpool.tile([128, 128], F16, tag=f"hs_{b}", name=f"hs_{b}")
                   for b in range(nb)]
            C2t = [cpool.tile([128, 128], F16, tag=f"c2_{b}", name=f"c2_{b}")
                   for b in range(nb)]
            for b in range(nb):
                nc.vector.memset(h0T[b][:], 0.0)

            idx_sb = cpool.tile([128, TS16], I16)
            nc.sync.dma_start(idx_sb[:], idx16_d[:, :])
            idx32_sb = cpool.tile([128, TC], mybir.dt.int32)
            nc.sync.dma_start(idx32_sb[:], idx32_d[:, :])
            dstrel_sb = cpool.tile([128, TC], F32)
            nc.sync.dma_start(dstrel_sb[:], dstrel_d[:, :])
            m3e_sb = cpool.tile([128, TC], F32)
            nc.sync.dma_start(m3e_sb[:], m3e_d[:, :])
            dinv_sb = cpool.tile([128, nb], F32)
            nc.sync.dma_start(dinv_sb[:], dinv_d[:, :])
            c2_sb = cpool.tile([128, nb], F32)
            nc.sync.dma_start(c2_sb[:], c2_d[:, :])

            W1_sb = cpool.tile([128, KI * H], F16)
            nc.sync.dma_start(W1_sb[:], W1[:, :])
            b1_sb = cpool.tile([128, 1], F32)
            nc.sync.dma_start(b1_sb[:], b1c[:, :])
            WrT_sb = cpool.tile([128, L * H], F16)
            nc.sync.dma_start(WrT_sb[:], WrT[:, :])
            W2_sb = cpool.tile([H, DO], F16)
            nc.sync.dma_start(W2_sb[:], W2[:, :])
            b2_sb = cpool.tile([1, DO], F16)
            nc.sync.dma_start(b2_sb[:], b2r[:, :])
            ones_sb = cpool.tile([1, 128], F16)
            nc.vector.memset(ones_sb[:], 1.0)
            iota_sb = cpool.tile([128, 128], F16)
            nc.sync.dma_start(iota_sb[:], iota_d[:, :])
            ident = cpool.tile([128, 128], F16)
            nc.sync.dma_start(ident[:], ident_d[:, :])
            for b in range(nb):
                nc.vector.tensor_scalar(C2t[b][:], ident[:],
                                        c2_sb[:, b:b + 1], None, op0=ALU.mult)

            # ---- lin1: h0T = relu(W1.T @ xT + b1) ----
            for b in range(nb):
                rows = rows_of(b)
                ps = ph_pool.tile([128, 128], F32, tag="ph")
                for k in range(KI):
                    xs = work.tile([128, 128], F16, tag="xs")
                    nc.sync.dma_start(
                        xs[:, :rows],
                        xT[:, k * nsh + b * 128:k * nsh + b * 128 + rows])
                    nc.tensor.matmul(ps[:, :rows], W1_sb[:, k * H:(k + 1) * H],
                                     xs[:, :rows],
                                     start=(k == 0), stop=(k == KI - 1))
                nc.scalar.activation(h0T[b][:, :rows], ps[:, :rows], AF.Relu,
                                     bias=b1_sb[:, 0:1])

            # ---- layers ----
            for l in range(L):
                bounceA = dram.tile([hA, H], F16, tag="bA")
                bounceB = dram.tile([cfg.half_b, H], F16, tag="bB")
                tableA = dram.tile([rowsA, H], F16, tag="tA")
                tableB = dram.tile([rowsB, H], F16, tag="tB")
                for b in cfg.ord_blocks:
                    rows = rows_of(b)
                    hb = work.tile([128, 128], F16, tag="hb")
                    nc.scalar.activation(hb[:], h0T[b][:], AF.Relu)
                    ph = ph_pool.tile([128, 128], F32, tag="ph")
                    nc.tensor.matmul(ph[:], WrT_sb[:, l * H:(l + 1) * H],
                                     hb[:], start=True, stop=True)
                    ht = work.tile([128, 128], F16, tag="ht")
                    nc.scalar.activation(ht[:], ph[:], AF.Copy)
                    pt = pt_pool.tile([128, 128], F16, tag="pt")
                    nc.tensor.transpose(pt[:], ht[:], ident[:])
                    nc.scalar.activation(hsb[b][:], pt[:], AF.Copy,
                                         scale=dinv_sb[:, b:b + 1])
                    if b >= cfg.nbB:
                        r0 = (b - cfg.nbB) * 128
                        nc.sync.dma_start(bounceA[r0:r0 + rows, :],
                                          hsb[b][:rows, :])
                        if b == nb - 1:
                            nc.gpsimd.collective_compute(
                                "AllGather", ALU.bypass,
                                replica_groups=[list(range(P))],
                                ins=[bounceA.opt()], outs=[tableA.opt()])
                    else:
                        r0 = b * 128
                        nc.sync.dma_start(bounceB[r0:r0 + rows, :],
                                          hsb[b][:rows, :])
                        if b == cfg.nbB - 1:
                            nc.gpsimd.collective_compute(
                                "AllGather", ALU.bypass,
                                replica_groups=[list(range(P))],
                                ins=[bounceB.opt()], outs=[tableB.opt()])

                tbl = {0: tableA.opt(), 1: tableB.opt()}
                gp = {0: ga_pool, 1: gb_pool}
                for si, (blocks, entries) in enumerate(plan.sg_entries):
                    bufs = {}
                    base_chunk = {}
                    for (g, nch, ch0) in entries:
                        if nch == 0:
                            continue
                        mb = gp[g].tile([128, nch, H], F16, tag=f"g{g}",
                                        name=f"mb{g}_{si}")
                        if cfg.use_dma_gather:
                            CMAX = 48      # chunks per dma_gather call
                            for t0 in range(0, nch, CMAX):
                                tn = min(CMAX, nch - t0)
                                c0 = ch0 + t0
                                nc.gpsimd.dma_gather(
                                    mb[:, t0:t0 + tn, :], tbl[g],
                                    idx_sb[:, c0 * 8:(c0 + tn) * 8],
                                    num_idxs=tn * 128, num_idxs_reg=tn * 128,
                                    elem_size=H, queue_num=0,
                                    single_packet=False)
                        else:
                            for q0 in range(nch):
                                nc.gpsimd.indirect_dma_start(
                                    mb[:, q0, :], None, tbl[g],
                                    bass.IndirectOffsetOnAxis(
                                        ap=idx32_sb[:, ch0 + q0:ch0 + q0 + 1],
                                        axis=0))
                        bufs[g] = mb
                        base_chunk[g] = ch0
                    for b in blocks:
                        nch_tot = int(plan.nchunks[b, 0] + plan.nchunks[b, 1])
                        pa = pagg_pool.tile([128, 128], F32, tag="pagg")
                        nc.tensor.matmul(pa[:], hsb[b][:], C2t[b][:],
                                         start=True, stop=(nch_tot == 0))
                        done = 0
                        for g in (0, 1):
                            nch_bg = int(plan.nchunks[b, g])
                            if nch_bg == 0:
                                continue
                            bi = blocks.index(b)
                            loc = sum(int(plan.nchunks[bb, g])
                                      for bb in blocks[:bi])
                            mb = bufs[g]
                            for t in range(nch_bg):
                                c = base_chunk[g] + loc + t
                                sg_t = sgen_pool.tile([128, 128], F16,
                                                      tag="sg")
                                nc.vector.tensor_scalar(
                                    sg_t[:], iota_sb[:],
                                    dstrel_sb[:, c:c + 1],
                                    m3e_sb[:, c:c + 1],
                                    op0=ALU.is_equal, op1=ALU.mult)
                                done += 1
                                nc.tensor.matmul(pa[:], mb[:, loc + t, :],
                                                 sg_t[:], start=False,
                                                 stop=(done == nch_tot))
                        nc.vector.scalar_tensor_tensor(
                            h0T[b][:], h0T[b][:], gates[l], pa[:],
                            op0=ALU.mult, op1=ALU.add)

            # ---- lin2 ----
            for b in range(nb):
                rows = rows_of(b)
                hb2 = work.tile([128, 128], F16, tag="hb")
                nc.scalar.activation(hb2[:], h0T[b][:], AF.Relu)
                po = ph_pool.tile([128, DO], F32, tag="ph", name="po")
                nc.tensor.matmul(po[:rows, :], hb2[:, :rows], W2_sb[:, :],
                                 start=True, stop=False)
                nc.tensor.matmul(po[:rows, :], ones_sb[:, :rows], b2_sb[:],
                                 start=False, stop=True)
                ot = work.tile([128, DO], F32, tag="ot")
                nc.scalar.activation(ot[:rows, :], po[:rows, :], AF.Copy)
                nc.sync.dma_start(out_d[b * 128:b * 128 + rows, :],
                                  ot[:rows, :])

    nc.finalize()
    return nc


def make_in_maps(plan: Plan, x, W1, b1, Wr, W2, b2):
    cfg = plan.cfg
    nsh = cfg.nsh
    KI = cfg.d_in // 128
    W1m = np.ascontiguousarray(
        np.asarray(W1, np.float32).reshape(KI, 128, cfg.d_h)
        .transpose(1, 0, 2).reshape(128, KI * cfg.d_h)).astype(np.float16)
    # WrT[k, l*H+j] = Wr[l, j, k]
    WrTm = np.ascontiguousarray(
        np.asarray(Wr, np.float32).transpose(2, 0, 1)
        .reshape(128, -1)).astype(np.float16)
    iota_in = np.ascontiguousarray(
        np.tile(np.arange(128, dtype=np.float16)[None, :], (128, 1)))
    ident_in = np.eye(128, dtype=np.float16)
    common = dict(
        W1=W1m,
        b1c=np.ascontiguousarray(np.asarray(b1, np.float32).reshape(-1, 1)),
        WrT=WrTm,
        W2=np.ascontiguousarray(np.asarray(W2, np.float32)).astype(np.float16),
        b2r=np.ascontiguousarray(
            np.asarray(b2, np.float32).reshape(1, -1)).astype(np.float16),
        iota_in=iota_in, ident_in=ident_in,
    )
    in_maps = []
    x = np.asarray(x, np.float32)
    for r in range(cfg.n_cores):
        pc = plan.per_core[r]
        # xT[p, k*nsh + c] = x[c, k*128 + p]
        xT = np.ascontiguousarray(
            x[r * nsh:(r + 1) * nsh].T
            .reshape(KI, 128, nsh).transpose(1, 0, 2)
            .reshape(128, KI * nsh)).astype(np.float16)
        m = dict(common)
        m.update(
            xT=xT, idx16=pc["idx16"], idx32=pc["idx32"],
            dstrel=pc["dstrel"], m3e=pc["m3e"],
            dinv_cols=pc["dinv_cols"], c2_cols=pc["c2_cols"],
        )
        in_maps.append(m)
    return in_maps


_cache = {}


def kernel(x, W1, b1, Wr, eps, W2, b2, edge_index, *, trace=False, cfg=None):
    cfg = cfg or Cfg()
    x = np.asarray(x)
    edge_index = np.asarray(edge_index)
    gates = [float(1.0 + math.tanh(float(e))) for e in np.asarray(eps)]

    ck = hash((edge_index.tobytes(), tuple(gates), cfg.n, cfg.n_cores,
               cfg.sg_blocks, cfg.use_dma_gather))
    if ck not in _cache:
        plan = preprocess(edge_index, cfg)
        nc = build_bass(plan, gates)
        _cache.clear()
        _cache[ck] = (plan, nc)
    plan, nc = _cache[ck]

    in_maps = make_in_maps(plan, x, W1, b1, Wr, W2, b2)
    try:
        res = bass_utils.run_bass_kernel_spmd(
            nc, in_maps, core_ids=list(range(cfg.n_cores)), trace=trace)
    except ModuleNotFoundError:
        # axon NTFF profiling hook unavailable in this container
        res = bass_utils.run_bass_kernel_spmd(
            nc, in_maps, core_ids=list(range(cfg.n_cores)), trace=False)
    out = np.concatenate([r["out"] for r in res.results], axis=0)
    kernel.last_results = res
    return out.astype(np.float32)


# revision 11
# speedup vs baseline: 2.4446x; 1.7962x over previous
"""EnergyGCN Trainium2 kernel: 8-core SPMD Bass/Tile implementation.

Strategy (node sharding):
  - 50000 nodes sharded contiguously across 8 cores (6250 rows, 49 blocks of
    128 per core).  Hidden state h0 kept node-major in SBUF: h0[b] =
    [128 nodes, H].
  - Per layer l: per block: hb = relu(h0) (fp16), PE-transpose -> hT [H, d],
    hr = hT.T @ WrT (PSUM, node-major), scaled by dinv on the PSUM->SBUF
    copy: hsb[b] = dinv * hr (= h~, fp16, node-major).  hsb is DMA'd into
    one of two DRAM bounce halves (blocks nbB..nb-1 -> half A first, then
    0..nbB-1 -> half B) and each half AllGather'd into a [8*half, H] fp16
    table (Shared scratchpad output).
  - Edge messages h~[col] are fetched with batched gpsimd.dma_gather (int16
    row indices into the half tables, round-robin over 4 SWDGE queues) and
    aggregated on TensorE with one-hot segment matrices generated by DVE in
    batches of 8 chunks: S[e, d] = (dstrel_e == iota_d), so PSUM accumulates
        pa[d, h] = sum_e S[e,d] * msg[e,h] + sum_d' Is2[d',d] * hsb[d',h]
    with Is2 = diag(-2*deg_d + 1).  Epilogue: the -3*dinv_d dest scale rides
    the ACT PSUM->SBUF copy (per-partition scale), then DVE:
        h0 = gate_l * h0 + (-3*dinv) * pa
    which equals gate*h0 + 6*hr - 3*ahat(hr) with self-loops folded in.
  - relu applied on read (next layer / final lin2).

Reference math:
    h = relu(x @ W1 + b1); h0 = h
    for l: hr = h @ Wr[l].T ; hn = 6hr - 3*ahat(hr)
           h0 = (1+tanh(eps[l]))*h0 + hn ; h = relu(h0)
    out = h @ W2 + b2
with ahat(y) = segment_sum(w[:,None]*y[col], row, n), w = dinv[row]*dinv[col],
self-loops appended, deg = counts of row (incl self), dinv = rsqrt(deg).
"""

import math

import numpy as np

import concourse.bacc as bacc
import concourse.bass as bass
import concourse.mybir as mybir
import concourse.tile as tile
from concourse import bass_utils

F32 = mybir.dt.float32
F16 = mybir.dt.float16
I16 = mybir.dt.int16
AF = mybir.ActivationFunctionType
ALU = mybir.AluOpType

N_NODES = 50000
D_IN, D_H, D_OUT = 256, 128, 64
N_LAYERS = 4
N_CORES = 8


class Cfg:
    def __init__(self, n=N_NODES, n_cores=N_CORES, d_in=D_IN, d_h=D_H,
                 d_out=D_OUT, n_layers=N_LAYERS, sg_blocks=6, gather_bufs=2,
                 sgen_batch=8, n_queues=4, cmax=48, shared_tables=True):
        self.n = n
        self.n_cores = n_cores
        self.d_in = d_in
        self.d_h = d_h
        self.d_out = d_out
        self.n_layers = n_layers
        self.sg_blocks = sg_blocks
        self.gather_bufs = gather_bufs
        self.sgen_batch = sgen_batch
        self.n_queues = n_queues
        self.cmax = cmax
        self.shared_tables = shared_tables
        assert n % n_cores == 0
        self.nsh = n // n_cores
        self.nb = (self.nsh + 127) // 128
        # half B = blocks [0, nbB), allgathered second; half A first
        self.nbB = (self.nb + 1) // 2
        self.half_b = 128 * self.nbB
        self.ord_blocks = list(range(self.nbB, self.nb)) + list(range(self.nbB))


class Plan:
    pass


def preprocess(edge_index: np.ndarray, cfg: Cfg) -> Plan:
    n, P, nsh, nb = cfg.n, cfg.n_cores, cfg.nsh, cfg.nb
    row = edge_index[0].astype(np.int64)
    col = edge_index[1].astype(np.int64)
    deg = np.bincount(row, minlength=n).astype(np.float64) + 1.0  # + self loop
    dinv = 1.0 / np.sqrt(deg)
    m3 = (-3.0 * dinv).astype(np.float32)           # -3*dinv (dest scale)
    is2 = (-2.0 * deg + 1.0).astype(np.float32)     # identity inject diag

    # source node -> (group, row in half table);  g=0: half A (off>=half_b)
    HB = cfg.half_b
    s = col // nsh
    off = col - s * nsh
    hA = nsh - HB
    g_of = (off < HB).astype(np.int64)              # g=0 -> half A, g=1 -> B
    rowg = np.where(off >= HB, s * hA + off - HB, s * HB + off)

    ordpos = np.zeros(nb, dtype=np.int64)           # block -> position
    for i, b in enumerate(cfg.ord_blocks):
        ordpos[b] = i
    SGB = cfg.sg_blocks

    core = row // nsh
    per_core_edges = []
    counts = np.zeros((P, nb, 2), dtype=np.int64)
    for r in range(P):
        m = core == r
        rr = row[m] - r * nsh
        cc = rowg[m]
        g = g_of[m]
        blk = rr // 128
        pos = ordpos[blk]
        order = np.lexsort((rr, pos, g, pos // SGB))
        per_core_edges.append((rr[order], cc[order], g[order], blk[order]))
        np.add.at(counts[r], (blk, g), 1)

    nchunks = (counts.max(axis=0) + 127) // 128     # [nb, 2]

    # chunk layout: supergroups of SGB blocks in ord_blocks order; per sg:
    # (g0 chunks for its blocks in order) then (g1 chunks).
    slot_off = {}
    sg_entries = []
    offc = 0
    for i0 in range(0, nb, SGB):
        blocks = cfg.ord_blocks[i0:i0 + SGB]
        entries = []
        for g in (0, 1):
            c0 = offc
            for b in blocks:
                slot_off[(b, g)] = offc
                offc += int(nchunks[b, g]) * 128
            entries.append((g, (offc - c0) // 128, c0 // 128))
        sg_entries.append((blocks, entries))
    total_slots = offc
    total_chunks = total_slots // 128

    per_core = []
    for r in range(P):
        rr, cc, g, blk = per_core_edges[r]
        idx = np.zeros(total_slots, dtype=np.int16)     # pad -> row 0 (valid)
        dstrel = np.full(total_slots, -1.0, dtype=np.float16)
        for b in range(nb):
            for gg in (0, 1):
                msel = (blk == b) & (g == gg)
                k = int(msel.sum())
                if k == 0:
                    continue
                o = slot_off[(b, gg)]
                idx[o:o + k] = cc[msel]
                dstrel[o:o + k] = (rr[msel] - b * 128).astype(np.float16)
        # dma_gather index wrapping: slot i -> [i % 16, i // 16]; the Q7
        # tx/rx cores read different partition groups -> replicate to 128.
        idx16 = np.ascontiguousarray(np.tile(idx.reshape(-1, 16).T, (8, 1)))
        dstrel128 = np.ascontiguousarray(
            dstrel.reshape(total_chunks, 128).T)        # [128, nchunk] fp16

        def colmat(v):
            out = np.zeros((nb * 128,), dtype=np.float64)
            out[:nsh] = v
            return np.ascontiguousarray(
                out.reshape(nb, 128).T.astype(np.float32))

        per_core.append(dict(
            idx16=idx16,
            dstrel=dstrel128,
            dinv_cols=colmat(dinv[r * nsh:(r + 1) * nsh]),
            m3_cols=colmat(m3[r * nsh:(r + 1) * nsh]),
            is2_cols=colmat(is2[r * nsh:(r + 1) * nsh]),
        ))

    plan = Plan()
    plan.cfg = cfg
    plan.nchunks = nchunks
    plan.sg_entries = sg_entries
    plan.total_chunks = total_chunks
    plan.total_slots = total_slots
    plan.per_core = per_core
    return plan


def build_bass(plan: Plan, gates):
    cfg = plan.cfg
    nsh, nb, P = cfg.nsh, cfg.nb, cfg.n_cores
    H, DI, DO, L = cfg.d_h, cfg.d_in, cfg.d_out, cfg.n_layers
    TC = plan.total_chunks
    TS16 = plan.total_slots // 16
    SB = cfg.sgen_batch
    hA = nsh - cfg.half_b
    rowsA, rowsB = P * hA, P * cfg.half_b

    nc = bacc.Bacc("TRN2", target_bir_lowering=False, debug=False,
                   num_devices=P, num_swdge_queues=cfg.n_queues)

    KI = DI // 128
    xT = nc.dram_tensor("xT", [128, KI * nsh], F16, kind="ExternalInput")
    W1 = nc.dram_tensor("W1", [128, KI * H], F16, kind="ExternalInput")
    b1r = nc.dram_tensor("b1r", [1, H], F16, kind="ExternalInput")
    WrT = nc.dram_tensor("WrT", [128, L * H], F16, kind="ExternalInput")
    W2 = nc.dram_tensor("W2", [H, DO], F16, kind="ExternalInput")
    b2r = nc.dram_tensor("b2r", [1, DO], F16, kind="ExternalInput")
    idx16_d = nc.dram_tensor("idx16", [128, TS16], I16, kind="ExternalInput")
    dstrel_d = nc.dram_tensor("dstrel", [128, TC], F16, kind="ExternalInput")
    dinv_d = nc.dram_tensor("dinv_cols", [128, nb], F32, kind="ExternalInput")
    m3_d = nc.dram_tensor("m3_cols", [128, nb], F32, kind="ExternalInput")
    is2_d = nc.dram_tensor("is2_cols", [128, nb], F32, kind="ExternalInput")
    iota_d = nc.dram_tensor("iota_in", [128, SB * 128], F16,
                            kind="ExternalInput")
    ident_d = nc.dram_tensor("ident_in", [128, 128], F16, kind="ExternalInput")
    out_d = nc.dram_tensor("out", [nsh, DO], F32, kind="ExternalOutput")

    last_rows = nsh - (nb - 1) * 128

    def rows_of(b):
        return last_rows if b == nb - 1 else 128

    addr_space = "Shared" if cfg.shared_tables else "Local"

    with tile.TileContext(nc) as tc:
        with (
            tc.tile_pool(name="const", bufs=1) as cpool,
            tc.tile_pool(name="work", bufs=4) as work,
            tc.tile_pool(name="sgen", bufs=4) as sgen_pool,
            tc.tile_pool(name="ga", bufs=cfg.gather_bufs) as ga_pool,
            tc.tile_pool(name="gb", bufs=cfg.gather_bufs) as gb_pool,
            tc.tile_pool(name="pt", bufs=2, space="PSUM") as pt_pool,
            tc.tile_pool(name="ph", bufs=2, space="PSUM") as ph_pool,
            tc.tile_pool(name="pagg", bufs=3, space="PSUM") as pagg_pool,
            tc.tile_pool(name="dram", bufs=2, space="DRAM") as dram,
        ):
            # persistent per-block state (node-major)
            h0 = [cpool.tile([128, 128], F32, tag=f"h0_{b}", name=f"h0_{b}")
                  for b in range(nb)]
            hsb = [cpool.tile([128, 128], F16, tag=f"hs_{b}", name=f"hs_{b}")
                   for b in range(nb)]
            Is2t = [cpool.tile([128, 128], F16, tag=f"i2_{b}", name=f"i2_{b}")
                    for b in range(nb)]
            for b in range(nb):
                nc.vector.memset(h0[b][:], 0.0)

            idx_sb = cpool.tile([128, TS16], I16)
            nc.sync.dma_start(idx_sb[:], idx16_d[:, :])
            dstrel_sb = cpool.tile([128, TC], F16)
            nc.sync.dma_start(dstrel_sb[:], dstrel_d[:, :])
            dinv_sb = cpool.tile([128, nb], F32)
            nc.sync.dma_start(dinv_sb[:], dinv_d[:, :])
            m3_sb = cpool.tile([128, nb], F32)
            nc.sync.dma_start(m3_sb[:], m3_d[:, :])
            is2_sb = cpool.tile([128, nb], F32)
            nc.sync.dma_start(is2_sb[:], is2_d[:, :])

            W1_sb = cpool.tile([128, KI * H], F16)
            nc.sync.dma_start(W1_sb[:], W1[:, :])
            b1_sb = cpool.tile([1, H], F16)
            nc.sync.dma_start(b1_sb[:], b1r[:, :])
            WrT_sb = cpool.tile([128, L * H], F16)
            nc.sync.dma_start(WrT_sb[:], WrT[:, :])
            W2_sb = cpool.tile([H, DO], F16)
            nc.sync.dma_start(W2_sb[:], W2[:, :])
            b2_sb = cpool.tile([1, DO], F16)
            nc.sync.dma_start(b2_sb[:], b2r[:, :])
            ones_sb = cpool.tile([1, 128], F16)
            nc.vector.memset(ones_sb[:], 1.0)
            iota_sb = cpool.tile([128, SB * 128], F16)
            nc.sync.dma_start(iota_sb[:], iota_d[:, :])
            ident = cpool.tile([128, 128], F16)
            nc.sync.dma_start(ident[:], ident_d[:, :])
            for b in range(nb):
                nc.vector.tensor_scalar(Is2t[b][:], ident[:],
                                        is2_sb[:, b:b + 1], None, op0=ALU.mult)

            # ---- lin1: h0 = relu(x @ W1 + b1) (node-major) ----
            for b in range(nb):
                rows = rows_of(b)
                ps = ph_pool.tile([128, 128], F32, tag="ph")
                for k in range(KI):
                    xs = work.tile([128, 128], F16, tag="xs")
                    nc.sync.dma_start(
                        xs[:, :rows],
                        xT[:, k * nsh + b * 128:k * nsh + b * 128 + rows])
                    nc.tensor.matmul(ps[:rows, :], xs[:, :rows],
                                     W1_sb[:, k * H:(k + 1) * H],
                                     start=(k == 0), stop=False)
                nc.tensor.matmul(ps[:rows, :], ones_sb[:, :rows], b1_sb[:],
                                 start=False, stop=True)
                nc.scalar.activation(h0[b][:rows, :], ps[:rows, :], AF.Relu)

            # ---- layers ----
            qrr = [0]

            def next_q():
                q = qrr[0]
                qrr[0] = (q + 1) % cfg.n_queues
                return q

            for l in range(L):
                bounceA = dram.tile([hA, H], F16, tag="bA")
                bounceB = dram.tile([cfg.half_b, H], F16, tag="bB")
                tableA = dram.tile([rowsA, H], F16, tag="tA",
                                   addr_space=addr_space)
                tableB = dram.tile([rowsB, H], F16, tag="tB",
                                   addr_space=addr_space)
                for b in cfg.ord_blocks:
                    rows = rows_of(b)
                    hb = work.tile([128, 128], F16, tag="hb")
                    nc.scalar.activation(hb[:], h0[b][:], AF.Relu)
                    pt = pt_pool.tile([128, 128], F16, tag="pt")
                    nc.tensor.transpose(pt[:], hb[:], ident[:])
                    hT = work.tile([128, 128], F16, tag="hT")
                    nc.scalar.activation(hT[:], pt[:], AF.Copy)
                    ph = ph_pool.tile([128, 128], F32, tag="ph")
                    nc.tensor.matmul(ph[:], hT[:],
                                     WrT_sb[:, l * H:(l + 1) * H],
                                     start=True, stop=True)
                    nc.scalar.activation(hsb[b][:], ph[:], AF.Copy,
                                         scale=dinv_sb[:, b:b + 1])
                    if b >= cfg.nbB:
                        r0 = (b - cfg.nbB) * 128
                        nc.sync.dma_start(bounceA[r0:r0 + rows, :],
                                          hsb[b][:rows, :])
                        if b == nb - 1:
                            nc.gpsimd.collective_compute(
                                "AllGather", ALU.bypass,
                                replica_groups=[list(range(P))],
                                ins=[bounceA.opt()], outs=[tableA.opt()])
                    else:
                        r0 = b * 128
                        nc.sync.dma_start(bounceB[r0:r0 + rows, :],
                                          hsb[b][:rows, :])
                        if b == cfg.nbB - 1:
                            nc.gpsimd.collective_compute(
                                "AllGather", ALU.bypass,
                                replica_groups=[list(range(P))],
                                ins=[bounceB.opt()], outs=[tableB.opt()])

                tbl = {0: tableA.opt(), 1: tableB.opt()}
                gp = {0: ga_pool, 1: gb_pool}
                for si, (blocks, entries) in enumerate(plan.sg_entries):
                    bufs = {}
                    base_chunk = {}
                    for (g, nch, ch0) in entries:
                        if nch == 0:
                            continue
                        mb = gp[g].tile([128, nch, H], F16, tag=f"g{g}",
                                        name=f"mb{g}_{si}")
                        for t0 in range(0, nch, cfg.cmax):
                            tn = min(cfg.cmax, nch - t0)
                            c0 = ch0 + t0
                            nc.gpsimd.dma_gather(
                                mb[:, t0:t0 + tn, :], tbl[g],
                                idx_sb[:, c0 * 8:(c0 + tn) * 8],
                                num_idxs=tn * 128, num_idxs_reg=tn * 128,
                                elem_size=H, queue_num=next_q(),
                                single_packet=False)
                        bufs[g] = mb
                        base_chunk[g] = ch0
                    for b in blocks:
                        nch_tot = int(plan.nchunks[b, 0] + plan.nchunks[b, 1])
                        bi = blocks.index(b)
                        pa = pagg_pool.tile([128, 128], F32, tag="pagg")
                        nc.tensor.matmul(pa[:], Is2t[b][:], hsb[b][:],
                                         start=True, stop=(nch_tot == 0))
                        done = 0
                        for g in (0, 1):
                            nch_bg = int(plan.nchunks[b, g])
                            if nch_bg == 0:
                                continue
                            loc = sum(int(plan.nchunks[bb, g])
                                      for bb in blocks[:bi])
                            mb = bufs[g]
                            for t0 in range(0, nch_bg, SB):
                                tn = min(SB, nch_bg - t0)
                                c = base_chunk[g] + loc + t0
                                sg_t = sgen_pool.tile([128, SB * 128], F16,
                                                      tag="sg")
                                dsl = dstrel_sb[:, c:c + tn]
                                nc.vector.tensor_tensor(
                                    sg_t[:, :tn * 128].rearrange(
                                        "p (c d) -> p c d", d=128),
                                    dsl.unsqueeze(2).broadcast_to(
                                        [128, tn, 128]),
                                    iota_sb[:, :tn * 128].rearrange(
                                        "p (c d) -> p c d", d=128),
                                    op=ALU.is_equal)
                                for t in range(tn):
                                    done += 1
                                    nc.tensor.matmul(
                                        pa[:],
                                        sg_t[:, t * 128:(t + 1) * 128],
                                        mb[:, loc + t0 + t, :],
                                        start=False, stop=(done == nch_tot))
                        hm = work.tile([128, 128], F32, tag="hm")
                        nc.scalar.activation(hm[:], pa[:], AF.Copy,
                                             scale=m3_sb[:, b:b + 1])
                        nc.vector.scalar_tensor_tensor(
                            h0[b][:], h0[b][:], gates[l], hm[:],
                            op0=ALU.mult, op1=ALU.add)

            # ---- lin2 ----
            for b in range(nb):
                rows = rows_of(b)
                hb = work.tile([128, 128], F16, tag="hb")
                nc.scalar.activation(hb[:], h0[b][:], AF.Relu)
                pt = pt_pool.tile([128, 128], F16, tag="pt")
                nc.tensor.transpose(pt[:], hb[:], ident[:])
                hT = work.tile([128, 128], F16, tag="hT")
                nc.scalar.activation(hT[:], pt[:], AF.Copy)
                po = ph_pool.tile([128, DO], F32, tag="ph", name="po")
                nc.tensor.matmul(po[:rows, :], hT[:, :rows], W2_sb[:, :],
                                 start=True, stop=False)
                nc.tensor.matmul(po[:rows, :], ones_sb[:, :rows], b2_sb[:],
                                 start=False, stop=True)
                ot = work.tile([128, DO], F32, tag="ot")
                nc.scalar.activation(ot[:rows, :], po[:rows, :], AF.Copy)
                nc.sync.dma_start(out_d[b * 128:b * 128 + rows, :],
                                  ot[:rows, :])

    nc.finalize()
    return nc


def make_in_maps(plan: Plan, x, W1, b1, Wr, W2, b2):
    cfg = plan.cfg
    nsh = cfg.nsh
    KI = cfg.d_in // 128
    W1m = np.ascontiguousarray(
        np.asarray(W1, np.float32).reshape(KI, 128, cfg.d_h)
        .transpose(1, 0, 2).reshape(128, KI * cfg.d_h)).astype(np.float16)
    # WrT[k, l*H+j] = Wr[l, j, k]
    WrTm = np.ascontiguousarray(
        np.asarray(Wr, np.float32).transpose(2, 0, 1)
        .reshape(128, -1)).astype(np.float16)
    iota_in = np.ascontiguousarray(
        np.tile(np.arange(128, dtype=np.float16)[None, :],
                (128, cfg.sgen_batch)))
    ident_in = np.eye(128, dtype=np.float16)
    common = dict(
        W1=W1m,
        b1r=np.ascontiguousarray(
            np.asarray(b1, np.float32).reshape(1, -1)).astype(np.float16),
        WrT=WrTm,
        W2=np.ascontiguousarray(np.asarray(W2, np.float32)).astype(np.float16),
        b2r=np.ascontiguousarray(
            np.asarray(b2, np.float32).reshape(1, -1)).astype(np.float16),
        iota_in=iota_in, ident_in=ident_in,
    )
    in_maps = []
    x = np.asarray(x, np.float32)
    for r in range(cfg.n_cores):
        pc = plan.per_core[r]
        # xT[p, k*nsh + c] = x[c, k*128 + p]
        xTm = np.ascontiguousarray(
            x[r * nsh:(r + 1) * nsh].T
            .reshape(KI, 128, nsh).transpose(1, 0, 2)
            .reshape(128, KI * nsh)).astype(np.float16)
        m = dict(common)
        m.update(
            xT=xTm, idx16=pc["idx16"], dstrel=pc["dstrel"],
            dinv_cols=pc["dinv_cols"], m3_cols=pc["m3_cols"],
            is2_cols=pc["is2_cols"],
        )
        in_maps.append(m)
    return in_maps


_cache = {}


def kernel(x, W1, b1, Wr, eps, W2, b2, edge_index, *, trace=False, cfg=None):
    cfg = cfg or Cfg()
    x = np.asarray(x)
    edge_index = np.asarray(edge_index)
    gates = [float(1.0 + math.tanh(float(e))) for e in np.asarray(eps)]

    ck = hash((edge_index.tobytes(), tuple(gates), cfg.n, cfg.n_cores,
               cfg.sg_blocks, cfg.n_queues, cfg.cmax, cfg.shared_tables))
    if ck not in _cache:
        plan = preprocess(edge_index, cfg)
        nc = build_bass(plan, gates)
        _cache.clear()
        _cache[ck] = (plan, nc)
    plan, nc = _cache[ck]

    in_maps = make_in_maps(plan, x, W1, b1, Wr, W2, b2)
    try:
        res = bass_utils.run_bass_kernel_spmd(
            nc, in_maps, core_ids=list(range(cfg.n_cores)), trace=trace)
    except ModuleNotFoundError:
        # axon NTFF profiling hook unavailable in this container
        res = bass_utils.run_bass_kernel_spmd(
            nc, in_maps, core_ids=list(range(cfg.n_cores)), trace=False)
    out = np.concatenate([r["out"] for r in res.results], axis=0)
    kernel.last_results = res
    return out.astype(np.float32)


# revision 12
# speedup vs baseline: 2.5943x; 1.0613x over previous
"""EnergyGCN Trainium2 kernel: 8-core SPMD Bass/Tile implementation.

Strategy (node sharding):
  - 50000 nodes sharded contiguously across 8 cores (6250 rows, 49 blocks of
    128 per core).  Hidden state h0 kept node-major in SBUF: h0[b] =
    [128 nodes, H].
  - Per layer l: per block: hb = relu(h0) (fp16), PE-transpose -> hT [H, d],
    hr = hT.T @ WrT (PSUM, node-major), scaled by dinv on the PSUM->SBUF
    copy: hsb[b] = dinv * hr (= h~, fp16, node-major).  hsb is DMA'd into
    one of two DRAM bounce halves (blocks nbB..nb-1 -> half A first, then
    0..nbB-1 -> half B) and each half AllGather'd into a [8*half, H] fp16
    table (Shared scratchpad output).
  - Edge messages h~[col] are fetched with batched gpsimd.dma_gather (int16
    row indices into the half tables, round-robin over 4 SWDGE queues) and
    aggregated on TensorE with one-hot segment matrices generated by DVE in
    batches of 8 chunks: S[e, d] = (dstrel_e == iota_d), so PSUM accumulates
        pa[d, h] = sum_e S[e,d] * msg[e,h] + sum_d' Is2[d',d] * hsb[d',h]
    with Is2 = diag(-2*deg_d + 1).  Epilogue: the -3*dinv_d dest scale rides
    the ACT PSUM->SBUF copy (per-partition scale), then DVE:
        h0 = gate_l * h0 + (-3*dinv) * pa
    which equals gate*h0 + 6*hr - 3*ahat(hr) with self-loops folded in.
  - relu applied on read (next layer / final lin2).

Reference math:
    h = relu(x @ W1 + b1); h0 = h
    for l: hr = h @ Wr[l].T ; hn = 6hr - 3*ahat(hr)
           h0 = (1+tanh(eps[l]))*h0 + hn ; h = relu(h0)
    out = h @ W2 + b2
with ahat(y) = segment_sum(w[:,None]*y[col], row, n), w = dinv[row]*dinv[col],
self-loops appended, deg = counts of row (incl self), dinv = rsqrt(deg).
"""

import math

import numpy as np

import concourse.bacc as bacc
import concourse.bass as bass
import concourse.mybir as mybir
import concourse.tile as tile
from concourse import bass_utils

F32 = mybir.dt.float32
F16 = mybir.dt.float16
I16 = mybir.dt.int16
AF = mybir.ActivationFunctionType
ALU = mybir.AluOpType

N_NODES = 50000
D_IN, D_H, D_OUT = 256, 128, 64
N_LAYERS = 4
N_CORES = 8


class Cfg:
    def __init__(self, n=N_NODES, n_cores=N_CORES, d_in=D_IN, d_h=D_H,
                 d_out=D_OUT, n_layers=N_LAYERS, sg_blocks=6, gather_bufs=2,
                 sgen_batch=8, n_queues=4, cmax=48, shared_tables=True):
        self.n = n
        self.n_cores = n_cores
        self.d_in = d_in
        self.d_h = d_h
        self.d_out = d_out
        self.n_layers = n_layers
        self.sg_blocks = sg_blocks
        self.gather_bufs = gather_bufs
        self.sgen_batch = sgen_batch
        self.n_queues = n_queues
        self.cmax = cmax
        self.shared_tables = shared_tables
        assert n % n_cores == 0
        self.nsh = n // n_cores
        self.nb = (self.nsh + 127) // 128
        # half B = blocks [0, nbB), allgathered second; half A first
        self.nbB = (self.nb + 1) // 2
        self.half_b = 128 * self.nbB
        self.ord_blocks = list(range(self.nbB, self.nb)) + list(range(self.nbB))


class Plan:
    pass


def preprocess(edge_index: np.ndarray, cfg: Cfg) -> Plan:
    n, P, nsh, nb = cfg.n, cfg.n_cores, cfg.nsh, cfg.nb
    row = edge_index[0].astype(np.int64)
    col = edge_index[1].astype(np.int64)
    deg = np.bincount(row, minlength=n).astype(np.float64) + 1.0  # + self loop
    dinv = 1.0 / np.sqrt(deg)
    m3 = (-3.0 * dinv).astype(np.float32)           # -3*dinv (dest scale)
    is2 = (-2.0 * deg + 1.0).astype(np.float32)     # identity inject diag

    # source node -> (group, row in half table);  g=0: half A (off>=half_b)
    HB = cfg.half_b
    s = col // nsh
    off = col - s * nsh
    hA = nsh - HB
    g_of = (off < HB).astype(np.int64)              # g=0 -> half A, g=1 -> B
    rowg = np.where(off >= HB, s * hA + off - HB, s * HB + off)

    ordpos = np.zeros(nb, dtype=np.int64)           # block -> position
    for i, b in enumerate(cfg.ord_blocks):
        ordpos[b] = i
    SGB = cfg.sg_blocks

    core = row // nsh
    per_core_edges = []
    counts = np.zeros((P, nb, 2), dtype=np.int64)
    for r in range(P):
        m = core == r
        rr = row[m] - r * nsh
        cc = rowg[m]
        g = g_of[m]
        blk = rr // 128
        pos = ordpos[blk]
        order = np.lexsort((rr, pos, g, pos // SGB))
        per_core_edges.append((rr[order], cc[order], g[order], blk[order]))
        np.add.at(counts[r], (blk, g), 1)

    nchunks = (counts.max(axis=0) + 127) // 128     # [nb, 2]

    # chunk layout: supergroups of SGB blocks in ord_blocks order; per sg:
    # (g0 chunks for its blocks in order) then (g1 chunks).
    slot_off = {}
    sg_entries = []
    offc = 0
    for i0 in range(0, nb, SGB):
        blocks = cfg.ord_blocks[i0:i0 + SGB]
        entries = []
        for g in (0, 1):
            c0 = offc
            for b in blocks:
                slot_off[(b, g)] = offc
                offc += int(nchunks[b, g]) * 128
            entries.append((g, (offc - c0) // 128, c0 // 128))
        sg_entries.append((blocks, entries))
    total_slots = offc
    total_chunks = total_slots // 128

    per_core = []
    for r in range(P):
        rr, cc, g, blk = per_core_edges[r]
        idx = np.zeros(total_slots, dtype=np.int16)     # pad -> row 0 (valid)
        dstrel = np.full(total_slots, -1.0, dtype=np.float16)
        for b in range(nb):
            for gg in (0, 1):
                msel = (blk == b) & (g == gg)
                k = int(msel.sum())
                if k == 0:
                    continue
                o = slot_off[(b, gg)]
                idx[o:o + k] = cc[msel]
                dstrel[o:o + k] = (rr[msel] - b * 128).astype(np.float16)
        # dma_gather index wrapping: slot i -> [i % 16, i // 16]; the Q7
        # tx/rx cores read different partition groups -> replicate to 128.
        idx16 = np.ascontiguousarray(np.tile(idx.reshape(-1, 16).T, (8, 1)))
        dstrel128 = np.ascontiguousarray(
            dstrel.reshape(total_chunks, 128).T)        # [128, nchunk] fp16

        def colmat(v):
            out = np.zeros((nb * 128,), dtype=np.float64)
            out[:nsh] = v
            return np.ascontiguousarray(
                out.reshape(nb, 128).T.astype(np.float32))

        per_core.append(dict(
            idx16=idx16,
            dstrel=dstrel128,
            dinv_cols=colmat(dinv[r * nsh:(r + 1) * nsh]),
            m3_cols=colmat(m3[r * nsh:(r + 1) * nsh]),
            is2_cols=colmat(is2[r * nsh:(r + 1) * nsh]),
        ))

    plan = Plan()
    plan.cfg = cfg
    plan.nchunks = nchunks
    plan.sg_entries = sg_entries
    plan.total_chunks = total_chunks
    plan.total_slots = total_slots
    plan.per_core = per_core
    return plan


def build_bass(plan: Plan, gates):
    cfg = plan.cfg
    nsh, nb, P = cfg.nsh, cfg.nb, cfg.n_cores
    H, DI, DO, L = cfg.d_h, cfg.d_in, cfg.d_out, cfg.n_layers
    TC = plan.total_chunks
    TS16 = plan.total_slots // 16
    SB = cfg.sgen_batch
    hA = nsh - cfg.half_b
    rowsA, rowsB = P * hA, P * cfg.half_b

    nc = bacc.Bacc("TRN2", target_bir_lowering=False, debug=False,
                   num_devices=P, num_swdge_queues=cfg.n_queues)

    KI = DI // 128
    xT = nc.dram_tensor("xT", [128, KI * nsh], F16, kind="ExternalInput")
    W1 = nc.dram_tensor("W1", [128, KI * H], F16, kind="ExternalInput")
    b1r = nc.dram_tensor("b1r", [1, H], F16, kind="ExternalInput")
    WrT = nc.dram_tensor("WrT", [128, L * H], F16, kind="ExternalInput")
    W2 = nc.dram_tensor("W2", [H, DO], F16, kind="ExternalInput")
    b2r = nc.dram_tensor("b2r", [1, DO], F16, kind="ExternalInput")
    idx16_d = nc.dram_tensor("idx16", [128, TS16], I16, kind="ExternalInput")
    dstrel_d = nc.dram_tensor("dstrel", [128, TC], F16, kind="ExternalInput")
    dinv_d = nc.dram_tensor("dinv_cols", [128, nb], F32, kind="ExternalInput")
    m3_d = nc.dram_tensor("m3_cols", [128, nb], F32, kind="ExternalInput")
    is2_d = nc.dram_tensor("is2_cols", [128, nb], F32, kind="ExternalInput")
    iota_d = nc.dram_tensor("iota_in", [128, SB * 128], F16,
                            kind="ExternalInput")
    ident_d = nc.dram_tensor("ident_in", [128, 128], F16, kind="ExternalInput")
    out_d = nc.dram_tensor("out", [nsh, DO], F32, kind="ExternalOutput")

    last_rows = nsh - (nb - 1) * 128

    def rows_of(b):
        return last_rows if b == nb - 1 else 128

    addr_space = "Shared" if cfg.shared_tables else "Local"

    with tile.TileContext(nc) as tc:
        with (
            tc.tile_pool(name="const", bufs=1) as cpool,
            tc.tile_pool(name="work", bufs=4) as work,
            tc.tile_pool(name="sgen", bufs=4) as sgen_pool,
            tc.tile_pool(name="ga", bufs=cfg.gather_bufs) as ga_pool,
            tc.tile_pool(name="gb", bufs=cfg.gather_bufs) as gb_pool,
            tc.tile_pool(name="pt", bufs=2, space="PSUM") as pt_pool,
            tc.tile_pool(name="ph", bufs=2, space="PSUM") as ph_pool,
            tc.tile_pool(name="pagg", bufs=3, space="PSUM") as pagg_pool,
            tc.tile_pool(name="dram", bufs=2, space="DRAM") as dram,
        ):
            # persistent per-block state (node-major)
            h0 = [cpool.tile([128, 128], F32, tag=f"h0_{b}", name=f"h0_{b}")
                  for b in range(nb)]
            hsb = [cpool.tile([128, 128], F16, tag=f"hs_{b}", name=f"hs_{b}")
                   for b in range(nb)]
            Is2t = [cpool.tile([128, 128], F16, tag=f"i2_{b}", name=f"i2_{b}")
                    for b in range(nb)]
            for b in range(nb):
                nc.vector.memset(h0[b][:], 0.0)

            idx_sb = cpool.tile([128, TS16], I16)
            nc.sync.dma_start(idx_sb[:], idx16_d[:, :])
            dstrel_sb = cpool.tile([128, TC], F16)
            nc.sync.dma_start(dstrel_sb[:], dstrel_d[:, :])
            dinv_sb = cpool.tile([128, nb], F32)
            nc.sync.dma_start(dinv_sb[:], dinv_d[:, :])
            m3_sb = cpool.tile([128, nb], F32)
            nc.sync.dma_start(m3_sb[:], m3_d[:, :])
            is2_sb = cpool.tile([128, nb], F32)
            nc.sync.dma_start(is2_sb[:], is2_d[:, :])

            W1_sb = cpool.tile([128, KI * H], F16)
            nc.sync.dma_start(W1_sb[:], W1[:, :])
            b1_sb = cpool.tile([1, H], F16)
            nc.sync.dma_start(b1_sb[:], b1r[:, :])
            WrT_sb = cpool.tile([128, L * H], F16)
            nc.sync.dma_start(WrT_sb[:], WrT[:, :])
            W2_sb = cpool.tile([H, DO], F16)
            nc.sync.dma_start(W2_sb[:], W2[:, :])
            b2_sb = cpool.tile([1, DO], F16)
            nc.sync.dma_start(b2_sb[:], b2r[:, :])
            ones_sb = cpool.tile([1, 128], F16)
            nc.vector.memset(ones_sb[:], 1.0)
            iota_sb = cpool.tile([128, SB * 128], F16)
            nc.sync.dma_start(iota_sb[:], iota_d[:, :])
            ident = cpool.tile([128, 128], F16)
            nc.sync.dma_start(ident[:], ident_d[:, :])
            for b in range(nb):
                nc.vector.tensor_scalar(Is2t[b][:], ident[:],
                                        is2_sb[:, b:b + 1], None, op0=ALU.mult)

            # ---- lin1: h0 = relu(x @ W1 + b1) (node-major) ----
            for b in range(nb):
                rows = rows_of(b)
                ps = ph_pool.tile([128, 128], F32, tag="ph")
                for k in range(KI):
                    xs = work.tile([128, 128], F16, tag="xs")
                    nc.sync.dma_start(
                        xs[:, :rows],
                        xT[:, k * nsh + b * 128:k * nsh + b * 128 + rows])
                    nc.tensor.matmul(ps[:rows, :], xs[:, :rows],
                                     W1_sb[:, k * H:(k + 1) * H],
                                     start=(k == 0), stop=False)
                nc.tensor.matmul(ps[:rows, :], ones_sb[:, :rows], b1_sb[:],
                                 start=False, stop=True)
                nc.scalar.activation(h0[b][:rows, :], ps[:rows, :], AF.Relu)

            # ---- layers ----
            qrr = [0]

            def next_q():
                q = qrr[0]
                qrr[0] = (q + 1) % cfg.n_queues
                return q

            for l in range(L):
                bounceA = dram.tile([hA, H], F16, tag="bA")
                bounceB = dram.tile([cfg.half_b, H], F16, tag="bB")
                tableA = dram.tile([rowsA, H], F16, tag="tA",
                                   addr_space=addr_space)
                tableB = dram.tile([rowsB, H], F16, tag="tB",
                                   addr_space=addr_space)
                for b in cfg.ord_blocks:
                    rows = rows_of(b)
                    hb = work.tile([128, 128], F16, tag="hb")
                    nc.scalar.activation(hb[:], h0[b][:], AF.Relu)
                    pt = pt_pool.tile([128, 128], F16, tag="pt")
                    nc.tensor.transpose(pt[:], hb[:], ident[:])
                    hT = work.tile([128, 128], F16, tag="hT")
                    nc.scalar.activation(hT[:], pt[:], AF.Copy)
                    ph = ph_pool.tile([128, 128], F32, tag="ph")
                    nc.tensor.matmul(ph[:], hT[:],
                                     WrT_sb[:, l * H:(l + 1) * H],
                                     start=True, stop=True)
                    nc.scalar.activation(hsb[b][:], ph[:], AF.Copy,
                                         scale=dinv_sb[:, b:b + 1])
                    if b >= cfg.nbB:
                        r0 = (b - cfg.nbB) * 128
                        nc.sync.dma_start(bounceA[r0:r0 + rows, :],
                                          hsb[b][:rows, :])
                        if b == nb - 1:
                            nc.gpsimd.collective_compute(
                                "AllGather", ALU.bypass,
                                replica_groups=[list(range(P))],
                                ins=[bounceA.opt()], outs=[tableA.opt()])
                    else:
                        r0 = b * 128
                        nc.sync.dma_start(bounceB[r0:r0 + rows, :],
                                          hsb[b][:rows, :])
                        if b == cfg.nbB - 1:
                            nc.gpsimd.collective_compute(
                                "AllGather", ALU.bypass,
                                replica_groups=[list(range(P))],
                                ins=[bounceB.opt()], outs=[tableB.opt()])

                tbl = {0: tableA.opt(), 1: tableB.opt()}
                gp = {0: ga_pool, 1: gb_pool}
                # two-phase aggregation: phase g=0 (half A table, with the
                # Is2 inject and the gated epilogue), then phase g=1 adds the
                # half-B messages.  This keeps the PE busy on g0 work while
                # the half-B AllGather is still in flight.
                for g in (0, 1):
                    for si, (blocks, entries) in enumerate(plan.sg_entries):
                        ent = [e for e in entries if e[0] == g]
                        if not ent or ent[0][1] == 0:
                            mb = None
                        else:
                            (_, nch, ch0) = ent[0]
                            mb = gp[g].tile([128, nch, H], F16, tag=f"g{g}",
                                            name=f"mb{g}_{si}")
                            for t0 in range(0, nch, cfg.cmax):
                                tn = min(cfg.cmax, nch - t0)
                                c0 = ch0 + t0
                                nc.gpsimd.dma_gather(
                                    mb[:, t0:t0 + tn, :], tbl[g],
                                    idx_sb[:, c0 * 8:(c0 + tn) * 8],
                                    num_idxs=tn * 128, num_idxs_reg=tn * 128,
                                    elem_size=H, queue_num=next_q(),
                                    single_packet=False)
                        for b in blocks:
                            nch_bg = int(plan.nchunks[b, g])
                            bi = blocks.index(b)
                            if g == 0:
                                pa = pagg_pool.tile([128, 128], F32,
                                                    tag="pagg")
                                nc.tensor.matmul(pa[:], Is2t[b][:],
                                                 hsb[b][:],
                                                 start=True,
                                                 stop=(nch_bg == 0))
                            else:
                                if nch_bg == 0:
                                    continue
                                pa = pagg_pool.tile([128, 128], F32,
                                                    tag="pagg")
                            done = 0
                            loc = sum(int(plan.nchunks[bb, g])
                                      for bb in blocks[:bi])
                            for t0 in range(0, nch_bg, SB):
                                tn = min(SB, nch_bg - t0)
                                c = ent[0][2] + loc + t0
                                sg_t = sgen_pool.tile([128, SB * 128], F16,
                                                      tag="sg")
                                dsl = dstrel_sb[:, c:c + tn]
                                nc.vector.tensor_tensor(
                                    sg_t[:, :tn * 128].rearrange(
                                        "p (c d) -> p c d", d=128),
                                    dsl.unsqueeze(2).broadcast_to(
                                        [128, tn, 128]),
                                    iota_sb[:, :tn * 128].rearrange(
                                        "p (c d) -> p c d", d=128),
                                    op=ALU.is_equal)
                                for t in range(tn):
                                    done += 1
                                    nc.tensor.matmul(
                                        pa[:],
                                        sg_t[:, t * 128:(t + 1) * 128],
                                        mb[:, loc + t0 + t, :],
                                        start=(g == 1 and done == 1),
                                        stop=(done == nch_bg))
                            hm = work.tile([128, 128], F32, tag="hm")
                            nc.scalar.activation(hm[:], pa[:], AF.Copy,
                                                 scale=m3_sb[:, b:b + 1])
                            if g == 0:
                                nc.vector.scalar_tensor_tensor(
                                    h0[b][:], h0[b][:], gates[l], hm[:],
                                    op0=ALU.mult, op1=ALU.add)
                            else:
                                nc.vector.tensor_tensor(
                                    h0[b][:], h0[b][:], hm[:], op=ALU.add)

            # ---- lin2 ----
            for b in range(nb):
                rows = rows_of(b)
                hb = work.tile([128, 128], F16, tag="hb")
                nc.scalar.activation(hb[:], h0[b][:], AF.Relu)
                pt = pt_pool.tile([128, 128], F16, tag="pt")
                nc.tensor.transpose(pt[:], hb[:], ident[:])
                hT = work.tile([128, 128], F16, tag="hT")
                nc.scalar.activation(hT[:], pt[:], AF.Copy)
                po = ph_pool.tile([128, DO], F32, tag="ph", name="po")
                nc.tensor.matmul(po[:rows, :], hT[:, :rows], W2_sb[:, :],
                                 start=True, stop=False)
                nc.tensor.matmul(po[:rows, :], ones_sb[:, :rows], b2_sb[:],
                                 start=False, stop=True)
                ot = work.tile([128, DO], F32, tag="ot")
                nc.scalar.activation(ot[:rows, :], po[:rows, :], AF.Copy)
                nc.sync.dma_start(out_d[b * 128:b * 128 + rows, :],
                                  ot[:rows, :])

    nc.finalize()
    return nc


def make_in_maps(plan: Plan, x, W1, b1, Wr, W2, b2):
    cfg = plan.cfg
    nsh = cfg.nsh
    KI = cfg.d_in // 128
    W1m = np.ascontiguousarray(
        np.asarray(W1, np.float32).reshape(KI, 128, cfg.d_h)
        .transpose(1, 0, 2).reshape(128, KI * cfg.d_h)).astype(np.float16)
    # WrT[k, l*H+j] = Wr[l, j, k]
    WrTm = np.ascontiguousarray(
        np.asarray(Wr, np.float32).transpose(2, 0, 1)
        .reshape(128, -1)).astype(np.float16)
    iota_in = np.ascontiguousarray(
        np.tile(np.arange(128, dtype=np.float16)[None, :],
                (128, cfg.sgen_batch)))
    ident_in = np.eye(128, dtype=np.float16)
    common = dict(
        W1=W1m,
        b1r=np.ascontiguousarray(
            np.asarray(b1, np.float32).reshape(1, -1)).astype(np.float16),
        WrT=WrTm,
        W2=np.ascontiguousarray(np.asarray(W2, np.float32)).astype(np.float16),
        b2r=np.ascontiguousarray(
            np.asarray(b2, np.float32).reshape(1, -1)).astype(np.float16),
        iota_in=iota_in, ident_in=ident_in,
    )
    in_maps = []
    x = np.asarray(x, np.float32)
    for r in range(cfg.n_cores):
        pc = plan.per_core[r]
        # xT[p, k*nsh + c] = x[c, k*128 + p]
        xTm = np.ascontiguousarray(
            x[r * nsh:(r + 1) * nsh].T
            .reshape(KI, 128, nsh).transpose(1, 0, 2)
            .reshape(128, KI * nsh)).astype(np.float16)
        m = dict(common)
        m.update(
            xT=xTm, idx16=pc["idx16"], dstrel=pc["dstrel"],
            dinv_cols=pc["dinv_cols"], m3_cols=pc["m3_cols"],
            is2_cols=pc["is2_cols"],
        )
        in_maps.append(m)
    return in_maps


_cache = {}


def kernel(x, W1, b1, Wr, eps, W2, b2, edge_index, *, trace=False, cfg=None):
    cfg = cfg or Cfg()
    x = np.asarray(x)
    edge_index = np.asarray(edge_index)
    gates = [float(1.0 + math.tanh(float(e))) for e in np.asarray(eps)]

    ck = hash((edge_index.tobytes(), tuple(gates), cfg.n, cfg.n_cores,
               cfg.sg_blocks, cfg.n_queues, cfg.cmax, cfg.shared_tables))
    if ck not in _cache:
        plan = preprocess(edge_index, cfg)
        nc = build_bass(plan, gates)
        _cache.clear()
        _cache[ck] = (plan, nc)
    plan, nc = _cache[ck]

    in_maps = make_in_maps(plan, x, W1, b1, Wr, W2, b2)
    try:
        res = bass_utils.run_bass_kernel_spmd(
            nc, in_maps, core_ids=list(range(cfg.n_cores)), trace=trace)
    except ModuleNotFoundError:
        # axon NTFF profiling hook unavailable in this container
        res = bass_utils.run_bass_kernel_spmd(
            nc, in_maps, core_ids=list(range(cfg.n_cores)), trace=False)
    out = np.concatenate([r["out"] for r in res.results], axis=0)
    kernel.last_results = res
    return out.astype(np.float32)


# revision 14
# speedup vs baseline: 2.7740x; 1.0693x over previous
"""EnergyGCN Trainium2 kernel: 8-core SPMD Bass/Tile implementation.

Strategy (node sharding):
  - 50000 nodes sharded contiguously across 8 cores (6250 rows, 49 blocks of
    128 per core).  Hidden state h0 kept node-major in SBUF: h0[b] =
    [128 nodes, H].
  - Per layer l: per block: hb = relu(h0) (fp16), PE-transpose -> hT [H, d],
    hr = hT.T @ WrT (PSUM, node-major), scaled by dinv on the PSUM->SBUF
    copy: hsb[b] = dinv * hr (= h~, fp16, node-major).  hsb is DMA'd into
    one of two DRAM bounce halves (blocks nbB..nb-1 -> half A first, then
    0..nbB-1 -> half B) and each half AllGather'd into a [8*half, H] fp16
    table (Shared scratchpad output).
  - Edge messages h~[col] are fetched with batched gpsimd.dma_gather (int16
    row indices into the half tables, round-robin over 4 SWDGE queues) and
    aggregated on TensorE with one-hot segment matrices generated by DVE in
    batches of 8 chunks: S[e, d] = (dstrel_e == iota_d), so PSUM accumulates
        pa[d, h] = sum_e S[e,d] * msg[e,h] + sum_d' Is2[d',d] * hsb[d',h]
    with Is2 = diag(-2*deg_d + 1).  Epilogue: the -3*dinv_d dest scale rides
    the ACT PSUM->SBUF copy (per-partition scale), then DVE:
        h0 = gate_l * h0 + (-3*dinv) * pa
    which equals gate*h0 + 6*hr - 3*ahat(hr) with self-loops folded in.
  - relu applied on read (next layer / final lin2).

Reference math:
    h = relu(x @ W1 + b1); h0 = h
    for l: hr = h @ Wr[l].T ; hn = 6hr - 3*ahat(hr)
           h0 = (1+tanh(eps[l]))*h0 + hn ; h = relu(h0)
    out = h @ W2 + b2
with ahat(y) = segment_sum(w[:,None]*y[col], row, n), w = dinv[row]*dinv[col],
self-loops appended, deg = counts of row (incl self), dinv = rsqrt(deg).
"""

import math

import numpy as np

import concourse.bacc as bacc
import concourse.bass as bass
import concourse.mybir as mybir
import concourse.tile as tile
from concourse import bass_utils

F32 = mybir.dt.float32
F16 = mybir.dt.float16
I16 = mybir.dt.int16
AF = mybir.ActivationFunctionType
ALU = mybir.AluOpType

N_NODES = 50000
D_IN, D_H, D_OUT = 256, 128, 64
N_LAYERS = 4
N_CORES = 8


class Cfg:
    def __init__(self, n=N_NODES, n_cores=N_CORES, d_in=D_IN, d_h=D_H,
                 d_out=D_OUT, n_layers=N_LAYERS, sg_blocks=6, gather_bufs=2,
                 sgen_batch=8, n_queues=4, cmax=48, shared_tables=True,
                 direct_table=True):
        self.n = n
        self.n_cores = n_cores
        self.d_in = d_in
        self.d_h = d_h
        self.d_out = d_out
        self.n_layers = n_layers
        self.sg_blocks = sg_blocks
        self.gather_bufs = gather_bufs
        self.sgen_batch = sgen_batch
        self.n_queues = n_queues
        self.cmax = cmax
        self.shared_tables = shared_tables
        self.direct_table = direct_table
        assert n % n_cores == 0
        self.nsh = n // n_cores
        self.nb = (self.nsh + 127) // 128
        # half B = blocks [0, nbB), allgathered second; half A first
        self.nbB = (self.nb + 1) // 2
        self.half_b = 128 * self.nbB
        self.ord_blocks = list(range(self.nbB, self.nb)) + list(range(self.nbB))


class Plan:
    pass


def preprocess(edge_index: np.ndarray, cfg: Cfg) -> Plan:
    n, P, nsh, nb = cfg.n, cfg.n_cores, cfg.nsh, cfg.nb
    row = edge_index[0].astype(np.int64)
    col = edge_index[1].astype(np.int64)
    deg = np.bincount(row, minlength=n).astype(np.float64) + 1.0  # + self loop
    dinv = 1.0 / np.sqrt(deg)
    m3 = (-3.0 * dinv).astype(np.float32)           # -3*dinv (dest scale)
    is2 = (-2.0 * deg + 1.0).astype(np.float32)     # identity inject diag

    # source node -> (group, row in half table);  g=0: half A (off>=half_b)
    HB = cfg.half_b
    s = col // nsh
    off = col - s * nsh
    hA = nsh - HB
    g_of = (off < HB).astype(np.int64)              # g=0 -> half A, g=1 -> B
    rowg = np.where(off >= HB, s * hA + off - HB, s * HB + off)

    ordpos = np.zeros(nb, dtype=np.int64)           # block -> position
    for i, b in enumerate(cfg.ord_blocks):
        ordpos[b] = i
    SGB = cfg.sg_blocks

    core = row // nsh
    per_core_edges = []
    counts = np.zeros((P, nb, 2), dtype=np.int64)
    for r in range(P):
        m = core == r
        rr = row[m] - r * nsh
        cc = rowg[m]
        g = g_of[m]
        blk = rr // 128
        pos = ordpos[blk]
        order = np.lexsort((rr, pos, g, pos // SGB))
        per_core_edges.append((rr[order], cc[order], g[order], blk[order]))
        np.add.at(counts[r], (blk, g), 1)

    nchunks = (counts.max(axis=0) + 127) // 128     # [nb, 2]

    # chunk layout: supergroups of SGB blocks in ord_blocks order; per sg:
    # (g0 chunks for its blocks in order) then (g1 chunks).
    slot_off = {}
    sg_entries = []
    offc = 0
    for i0 in range(0, nb, SGB):
        blocks = cfg.ord_blocks[i0:i0 + SGB]
        entries = []
        for g in (0, 1):
            c0 = offc
            for b in blocks:
                slot_off[(b, g)] = offc
                offc += int(nchunks[b, g]) * 128
            entries.append((g, (offc - c0) // 128, c0 // 128))
        sg_entries.append((blocks, entries))
    total_slots = offc
    total_chunks = total_slots // 128

    per_core = []
    for r in range(P):
        rr, cc, g, blk = per_core_edges[r]
        idx = np.zeros(total_slots, dtype=np.int16)     # pad -> row 0 (valid)
        dstrel = np.full(total_slots, -1.0, dtype=np.float16)
        for b in range(nb):
            for gg in (0, 1):
                msel = (blk == b) & (g == gg)
                k = int(msel.sum())
                if k == 0:
                    continue
                o = slot_off[(b, gg)]
                idx[o:o + k] = cc[msel]
                dstrel[o:o + k] = (rr[msel] - b * 128).astype(np.float16)
        # dma_gather index wrapping: slot i -> [i % 16, i // 16]; the Q7
        # tx/rx cores read different partition groups -> replicate to 128.
        idx16 = np.ascontiguousarray(np.tile(idx.reshape(-1, 16).T, (8, 1)))
        dstrel128 = np.ascontiguousarray(
            dstrel.reshape(total_chunks, 128).T)        # [128, nchunk] fp16

        def colmat(v):
            out = np.zeros((nb * 128,), dtype=np.float64)
            out[:nsh] = v
            return np.ascontiguousarray(
                out.reshape(nb, 128).T.astype(np.float32))

        per_core.append(dict(
            idx16=idx16,
            dstrel=dstrel128,
            dinv_cols=colmat(dinv[r * nsh:(r + 1) * nsh]),
            m3_cols=colmat(m3[r * nsh:(r + 1) * nsh]),
            is2_cols=colmat(is2[r * nsh:(r + 1) * nsh]),
        ))

    plan = Plan()
    plan.cfg = cfg
    plan.nchunks = nchunks
    plan.sg_entries = sg_entries
    plan.total_chunks = total_chunks
    plan.total_slots = total_slots
    plan.per_core = per_core
    return plan


def build_bass(plan: Plan, gates):
    cfg = plan.cfg
    nsh, nb, P = cfg.nsh, cfg.nb, cfg.n_cores
    H, DI, DO, L = cfg.d_h, cfg.d_in, cfg.d_out, cfg.n_layers
    TC = plan.total_chunks
    TS16 = plan.total_slots // 16
    SB = cfg.sgen_batch
    hA = nsh - cfg.half_b
    rowsA, rowsB = P * hA, P * cfg.half_b

    nc = bacc.Bacc("TRN2", target_bir_lowering=False, debug=False,
                   num_devices=P, num_swdge_queues=cfg.n_queues)

    KI = DI // 128
    xT = nc.dram_tensor("xT", [128, KI * nsh], F16, kind="ExternalInput")
    W1 = nc.dram_tensor("W1", [128, KI * H], F16, kind="ExternalInput")
    b1r = nc.dram_tensor("b1r", [1, H], F16, kind="ExternalInput")
    WrT = nc.dram_tensor("WrT", [128, L * H], F16, kind="ExternalInput")
    W2 = nc.dram_tensor("W2", [H, DO], F16, kind="ExternalInput")
    b2r = nc.dram_tensor("b2r", [1, DO], F16, kind="ExternalInput")
    idx16_d = nc.dram_tensor("idx16", [128, TS16], I16, kind="ExternalInput")
    dstrel_d = nc.dram_tensor("dstrel", [128, TC], F16, kind="ExternalInput")
    dinv_d = nc.dram_tensor("dinv_cols", [128, nb], F32, kind="ExternalInput")
    m3_d = nc.dram_tensor("m3_cols", [128, nb], F32, kind="ExternalInput")
    is2_d = nc.dram_tensor("is2_cols", [128, nb], F32, kind="ExternalInput")
    iota_d = nc.dram_tensor("iota_in", [128, SB * 128], F16,
                            kind="ExternalInput")
    ident_d = nc.dram_tensor("ident_in", [128, 128], F16, kind="ExternalInput")
    out_d = nc.dram_tensor("out", [nsh, DO], F32, kind="ExternalOutput")

    last_rows = nsh - (nb - 1) * 128

    def rows_of(b):
        return last_rows if b == nb - 1 else 128

    addr_space = "Shared" if cfg.shared_tables else "Local"

    with tile.TileContext(nc) as tc:
        with (
            tc.tile_pool(name="const", bufs=1) as cpool,
            tc.tile_pool(name="work", bufs=4) as work,
            tc.tile_pool(name="sgen", bufs=4) as sgen_pool,
            tc.tile_pool(name="ga", bufs=cfg.gather_bufs) as ga_pool,
            tc.tile_pool(name="gb", bufs=cfg.gather_bufs) as gb_pool,
            tc.tile_pool(name="pt", bufs=2, space="PSUM") as pt_pool,
            tc.tile_pool(name="ph", bufs=2, space="PSUM") as ph_pool,
            tc.tile_pool(name="pagg", bufs=4, space="PSUM") as pagg_pool,
            tc.tile_pool(name="dram", bufs=2, space="DRAM") as dram,
        ):
            # persistent per-block state (node-major)
            h0 = [cpool.tile([128, 128], F32, tag=f"h0_{b}", name=f"h0_{b}")
                  for b in range(nb)]
            hsb = [cpool.tile([128, 128], F16, tag=f"hs_{b}", name=f"hs_{b}")
                   for b in range(nb)]
            Is2t = [cpool.tile([128, 128], F16, tag=f"i2_{b}", name=f"i2_{b}")
                    for b in range(nb)]
            for b in range(nb):
                nc.vector.memset(h0[b][:], 0.0)

            idx_sb = cpool.tile([128, TS16], I16)
            nc.sync.dma_start(idx_sb[:], idx16_d[:, :])
            dstrel_sb = cpool.tile([128, TC], F16)
            nc.sync.dma_start(dstrel_sb[:], dstrel_d[:, :])
            dinv_sb = cpool.tile([128, nb], F32)
            nc.sync.dma_start(dinv_sb[:], dinv_d[:, :])
            m3_sb = cpool.tile([128, nb], F32)
            nc.sync.dma_start(m3_sb[:], m3_d[:, :])
            is2_sb = cpool.tile([128, nb], F32)
            nc.sync.dma_start(is2_sb[:], is2_d[:, :])

            W1_sb = cpool.tile([128, KI * H], F16)
            nc.sync.dma_start(W1_sb[:], W1[:, :])
            b1_sb = cpool.tile([1, H], F16)
            nc.sync.dma_start(b1_sb[:], b1r[:, :])
            WrT_sb = cpool.tile([128, L * H], F16)
            nc.sync.dma_start(WrT_sb[:], WrT[:, :])
            W2_sb = cpool.tile([H, DO], F16)
            nc.sync.dma_start(W2_sb[:], W2[:, :])
            b2_sb = cpool.tile([1, DO], F16)
            nc.sync.dma_start(b2_sb[:], b2r[:, :])
            ones_sb = cpool.tile([1, 128], F16)
            nc.vector.memset(ones_sb[:], 1.0)
            iota_sb = cpool.tile([128, SB * 128], F16)
            nc.sync.dma_start(iota_sb[:], iota_d[:, :])
            ident = cpool.tile([128, 128], F16)
            nc.sync.dma_start(ident[:], ident_d[:, :])
            for b in range(nb):
                nc.vector.tensor_scalar(Is2t[b][:], ident[:],
                                        is2_sb[:, b:b + 1], None, op0=ALU.mult)

            # ---- lin1: h0 = relu(x @ W1 + b1) (node-major) ----
            for b in range(nb):
                rows = rows_of(b)
                ps = ph_pool.tile([128, 128], F32, tag="ph")
                for k in range(KI):
                    xs = work.tile([128, 128], F16, tag="xs")
                    nc.sync.dma_start(
                        xs[:, :rows],
                        xT[:, k * nsh + b * 128:k * nsh + b * 128 + rows])
                    nc.tensor.matmul(ps[:rows, :], xs[:, :rows],
                                     W1_sb[:, k * H:(k + 1) * H],
                                     start=(k == 0), stop=False)
                nc.tensor.matmul(ps[:rows, :], ones_sb[:, :rows], b1_sb[:],
                                 start=False, stop=True)
                nc.scalar.activation(h0[b][:rows, :], ps[:rows, :], AF.Relu)

            # ---- layers ----
            qrr = [0]

            def next_q():
                q = qrr[0]
                qrr[0] = (q + 1) % cfg.n_queues
                return q

            if cfg.direct_table:
                pidv = nc.sync.partition_id()
                offA_base = nc.sync.compute_val(pidv * (hA * H))
                offB_base = nc.sync.compute_val(pidv * (cfg.half_b * H))

            for l in range(L):
                if cfg.direct_table:
                    tableA = dram.tile([rowsA + P, H], F16, tag="tA",
                                       addr_space="Shared")
                    tableB = dram.tile([rowsB + P, H], F16, tag="tB",
                                       addr_space="Shared")
                    flagA = dram.tile([1, H], F16, tag="fA")
                    flagB = dram.tile([1, H], F16, tag="fB")
                else:
                    bounceA = dram.tile([hA, H], F16, tag="bA")
                    bounceB = dram.tile([cfg.half_b, H], F16, tag="bB")
                    tableA = dram.tile([rowsA, H], F16, tag="tA",
                                       addr_space=addr_space)
                    tableB = dram.tile([rowsB, H], F16, tag="tB",
                                       addr_space=addr_space)
                taap = tableA.opt()
                tbap = tableB.opt()
                for b in cfg.ord_blocks:
                    rows = rows_of(b)
                    hb = work.tile([128, 128], F16, tag="hb")
                    nc.scalar.activation(hb[:], h0[b][:], AF.Relu)
                    pt = pt_pool.tile([128, 128], F16, tag="pt")
                    nc.tensor.transpose(pt[:], hb[:], ident[:])
                    hT = work.tile([128, 128], F16, tag="hT")
                    nc.scalar.activation(hT[:], pt[:], AF.Copy)
                    ph = ph_pool.tile([128, 128], F32, tag="ph")
                    nc.tensor.matmul(ph[:], hT[:],
                                     WrT_sb[:, l * H:(l + 1) * H],
                                     start=True, stop=True)
                    nc.scalar.activation(hsb[b][:], ph[:], AF.Copy,
                                         scale=dinv_sb[:, b:b + 1])
                    in_half_a = b >= cfg.nbB
                    r0 = (b - cfg.nbB) * 128 if in_half_a else b * 128
                    if cfg.direct_table:
                        tap = taap if in_half_a else tbap
                        base = offA_base if in_half_a else offB_base
                        dst = bass.AP(tap.tensor, base + r0 * H,
                                      [[H, rows], [1, H]],
                                      dep_tracking_offset=0)
                        nc.sync.dma_start(dst, hsb[b][:rows, :])
                        last = (b == nb - 1) if in_half_a \
                            else (b == cfg.nbB - 1)
                        if last:
                            tbl_t, flag, rr0 = (
                                (taap, flagA, rowsA) if in_half_a
                                else (tbap, flagB, rowsB))
                            fsb = work.tile([1, H], F16, tag="flag")
                            nc.sync.dma_start(fsb[:], tbl_t[0:1, :])
                            nc.sync.dma_start(flag[0:1, :], fsb[:])
                            nc.gpsimd.collective_compute(
                                "AllGather", ALU.bypass,
                                replica_groups=[list(range(P))],
                                ins=[flag.opt()],
                                outs=[tbl_t[rr0:rr0 + P, :]])
                    else:
                        bounce = bounceA if in_half_a else bounceB
                        nc.sync.dma_start(bounce[r0:r0 + rows, :],
                                          hsb[b][:rows, :])
                        last = (b == nb - 1) if in_half_a \
                            else (b == cfg.nbB - 1)
                        if last:
                            bnc, tbl_t = ((bounceA, tableA) if in_half_a
                                          else (bounceB, tableB))
                            nc.gpsimd.collective_compute(
                                "AllGather", ALU.bypass,
                                replica_groups=[list(range(P))],
                                ins=[bnc.opt()], outs=[tbl_t.opt()])

                tbl = {0: tableA.opt(), 1: tableB.opt()}
                gp = {0: ga_pool, 1: gb_pool}
                for si, (blocks, entries) in enumerate(plan.sg_entries):
                    bufs = {}
                    base_chunk = {}
                    for (g, nch, ch0) in entries:
                        if nch == 0:
                            continue
                        mb = gp[g].tile([128, nch, H], F16, tag=f"g{g}",
                                        name=f"mb{g}_{si}")
                        for t0 in range(0, nch, cfg.cmax):
                            tn = min(cfg.cmax, nch - t0)
                            c0 = ch0 + t0
                            nc.gpsimd.dma_gather(
                                mb[:, t0:t0 + tn, :], tbl[g],
                                idx_sb[:, c0 * 8:(c0 + tn) * 8],
                                num_idxs=tn * 128, num_idxs_reg=tn * 128,
                                elem_size=H, queue_num=next_q(),
                                single_packet=False)
                        bufs[g] = mb
                        base_chunk[g] = ch0
                    for b in blocks:
                        nch_tot = int(plan.nchunks[b, 0] + plan.nchunks[b, 1])
                        bi = blocks.index(b)
                        pa = pagg_pool.tile([128, 128], F32, tag="pagg")
                        nc.tensor.matmul(pa[:], Is2t[b][:], hsb[b][:],
                                         start=True, stop=(nch_tot == 0))
                        done = 0
                        for g in (0, 1):
                            nch_bg = int(plan.nchunks[b, g])
                            if nch_bg == 0:
                                continue
                            loc = sum(int(plan.nchunks[bb, g])
                                      for bb in blocks[:bi])
                            mb = bufs[g]
                            for t0 in range(0, nch_bg, SB):
                                tn = min(SB, nch_bg - t0)
                                c = base_chunk[g] + loc + t0
                                sg_t = sgen_pool.tile([128, SB * 128], F16,
                                                      tag="sg")
                                dsl = dstrel_sb[:, c:c + tn]
                                nc.vector.tensor_tensor(
                                    sg_t[:, :tn * 128].rearrange(
                                        "p (c d) -> p c d", d=128),
                                    dsl.unsqueeze(2).broadcast_to(
                                        [128, tn, 128]),
                                    iota_sb[:, :tn * 128].rearrange(
                                        "p (c d) -> p c d", d=128),
                                    op=ALU.is_equal)
                                for t in range(tn):
                                    done += 1
                                    nc.tensor.matmul(
                                        pa[:],
                                        sg_t[:, t * 128:(t + 1) * 128],
                                        mb[:, loc + t0 + t, :],
                                        start=False, stop=(done == nch_tot))
                        hm = work.tile([128, 128], F32, tag="hm")
                        nc.scalar.activation(hm[:], pa[:], AF.Copy,
                                             scale=m3_sb[:, b:b + 1])
                        nc.vector.scalar_tensor_tensor(
                            h0[b][:], h0[b][:], gates[l], hm[:],
                            op0=ALU.mult, op1=ALU.add)

            # ---- lin2 ----
            for b in range(nb):
                rows = rows_of(b)
                hb = work.tile([128, 128], F16, tag="hb")
                nc.scalar.activation(hb[:], h0[b][:], AF.Relu)
                pt = pt_pool.tile([128, 128], F16, tag="pt")
                nc.tensor.transpose(pt[:], hb[:], ident[:])
                hT = work.tile([128, 128], F16, tag="hT")
                nc.scalar.activation(hT[:], pt[:], AF.Copy)
                po = ph_pool.tile([128, DO], F32, tag="ph", name="po")
                nc.tensor.matmul(po[:rows, :], hT[:, :rows], W2_sb[:, :],
                                 start=True, stop=False)
                nc.tensor.matmul(po[:rows, :], ones_sb[:, :rows], b2_sb[:],
                                 start=False, stop=True)
                ot = work.tile([128, DO], F32, tag="ot")
                nc.scalar.activation(ot[:rows, :], po[:rows, :], AF.Copy)
                nc.sync.dma_start(out_d[b * 128:b * 128 + rows, :],
                                  ot[:rows, :])

    nc.finalize()
    return nc


def make_in_maps(plan: Plan, x, W1, b1, Wr, W2, b2):
    cfg = plan.cfg
    nsh = cfg.nsh
    KI = cfg.d_in // 128
    W1m = np.ascontiguousarray(
        np.asarray(W1, np.float32).reshape(KI, 128, cfg.d_h)
        .transpose(1, 0, 2).reshape(128, KI * cfg.d_h)).astype(np.float16)
    # WrT[k, l*H+j] = Wr[l, j, k]
    WrTm = np.ascontiguousarray(
        np.asarray(Wr, np.float32).transpose(2, 0, 1)
        .reshape(128, -1)).astype(np.float16)
    iota_in = np.ascontiguousarray(
        np.tile(np.arange(128, dtype=np.float16)[None, :],
                (128, cfg.sgen_batch)))
    ident_in = np.eye(128, dtype=np.float16)
    common = dict(
        W1=W1m,
        b1r=np.ascontiguousarray(
            np.asarray(b1, np.float32).reshape(1, -1)).astype(np.float16),
        WrT=WrTm,
        W2=np.ascontiguousarray(np.asarray(W2, np.float32)).astype(np.float16),
        b2r=np.ascontiguousarray(
            np.asarray(b2, np.float32).reshape(1, -1)).astype(np.float16),
        iota_in=iota_in, ident_in=ident_in,
    )
    in_maps = []
    x = np.asarray(x, np.float32)
    for r in range(cfg.n_cores):
        pc = plan.per_core[r]
        # xT[p, k*nsh + c] = x[c, k*128 + p]
        xTm = np.ascontiguousarray(
            x[r * nsh:(r + 1) * nsh].T
            .reshape(KI, 128, nsh).transpose(1, 0, 2)
            .reshape(128, KI * nsh)).astype(np.float16)
        m = dict(common)
        m.update(
            xT=xTm, idx16=pc["idx16"], dstrel=pc["dstrel"],
            dinv_cols=pc["dinv_cols"], m3_cols=pc["m3_cols"],
            is2_cols=pc["is2_cols"],
        )
        in_maps.append(m)
    return in_maps


_cache = {}


def kernel(x, W1, b1, Wr, eps, W2, b2, edge_index, *, trace=False, cfg=None):
    cfg = cfg or Cfg()
    x = np.asarray(x)
    edge_index = np.asarray(edge_index)
    gates = [float(1.0 + math.tanh(float(e))) for e in np.asarray(eps)]

    ck = hash((edge_index.tobytes(), tuple(gates), cfg.n, cfg.n_cores,
               cfg.sg_blocks, cfg.n_queues, cfg.cmax, cfg.shared_tables,
               cfg.direct_table))
    if ck not in _cache:
        plan = preprocess(edge_index, cfg)
        nc = build_bass(plan, gates)
        _cache.clear()
        _cache[ck] = (plan, nc)
    plan, nc = _cache[ck]

    in_maps = make_in_maps(plan, x, W1, b1, Wr, W2, b2)
    try:
        res = bass_utils.run_bass_kernel_spmd(
            nc, in_maps, core_ids=list(range(cfg.n_cores)), trace=trace)
    except ModuleNotFoundError:
        # axon NTFF profiling hook unavailable in this container
        res = bass_utils.run_bass_kernel_spmd(
            nc, in_maps, core_ids=list(range(cfg.n_cores)), trace=False)
    out = np.concatenate([r["out"] for r in res.results], axis=0)
    kernel.last_results = res
    return out.astype(np.float32)


# revision 17
# speedup vs baseline: 3.0612x; 1.1035x over previous
"""EnergyGCN Trainium2 kernel: 8-core SPMD Bass/Tile implementation.

Strategy (node sharding):
  - 50000 nodes sharded contiguously across 8 cores (6250 rows, 49 blocks of
    128 per core).  Hidden state h0 kept node-major in SBUF: h0[b] =
    [128 nodes, H].
  - Per layer l: per block: hb = relu(h0) (fp16), PE-transpose -> hT [H, d],
    hr = hT.T @ WrT (PSUM, node-major), scaled by dinv on the PSUM->SBUF
    copy: hsb[b] = dinv * hr (= h~, fp16, node-major).  hsb is DMA'd into
    one of two DRAM bounce halves (blocks nbB..nb-1 -> half A first, then
    0..nbB-1 -> half B) and each half AllGather'd into a [8*half, H] fp16
    table (Shared scratchpad output).
  - Edge messages h~[col] are fetched with batched gpsimd.dma_gather (int16
    row indices into the half tables, round-robin over 4 SWDGE queues) and
    aggregated on TensorE with one-hot segment matrices generated by DVE in
    batches of 8 chunks: S[e, d] = (dstrel_e == iota_d), so PSUM accumulates
        pa[d, h] = sum_e S[e,d] * msg[e,h] + sum_d' Is2[d',d] * hsb[d',h]
    with Is2 = diag(-2*deg_d + 1).  Epilogue: the -3*dinv_d dest scale rides
    the ACT PSUM->SBUF copy (per-partition scale), then DVE:
        h0 = gate_l * h0 + (-3*dinv) * pa
    which equals gate*h0 + 6*hr - 3*ahat(hr) with self-loops folded in.
  - relu applied on read (next layer / final lin2).

Reference math:
    h = relu(x @ W1 + b1); h0 = h
    for l: hr = h @ Wr[l].T ; hn = 6hr - 3*ahat(hr)
           h0 = (1+tanh(eps[l]))*h0 + hn ; h = relu(h0)
    out = h @ W2 + b2
with ahat(y) = segment_sum(w[:,None]*y[col], row, n), w = dinv[row]*dinv[col],
self-loops appended, deg = counts of row (incl self), dinv = rsqrt(deg).
"""

import math

import numpy as np

import concourse.bacc as bacc
import concourse.bass as bass
import concourse.mybir as mybir
import concourse.tile as tile
from concourse import bass_utils

F32 = mybir.dt.float32
F16 = mybir.dt.float16
I16 = mybir.dt.int16
AF = mybir.ActivationFunctionType
ALU = mybir.AluOpType

N_NODES = 50000
D_IN, D_H, D_OUT = 256, 128, 64
N_LAYERS = 4
N_CORES = 8


class Cfg:
    def __init__(self, n=N_NODES, n_cores=N_CORES, d_in=D_IN, d_h=D_H,
                 d_out=D_OUT, n_layers=N_LAYERS, sg_blocks=6, gather_bufs=3,
                 sgen_batch=8, n_queues=4, cmax=14, shared_tables=True,
                 direct_table=False):
        self.n = n
        self.n_cores = n_cores
        self.d_in = d_in
        self.d_h = d_h
        self.d_out = d_out
        self.n_layers = n_layers
        self.sg_blocks = sg_blocks
        self.gather_bufs = gather_bufs
        self.sgen_batch = sgen_batch
        self.n_queues = n_queues
        self.cmax = cmax
        self.shared_tables = shared_tables
        self.direct_table = direct_table
        assert n % n_cores == 0
        self.nsh = n // n_cores
        self.nb = (self.nsh + 127) // 128
        # half B = blocks [0, nbB), allgathered second; half A first
        self.nbB = (self.nb + 1) // 2
        self.half_b = 128 * self.nbB
        self.ord_blocks = list(range(self.nbB, self.nb)) + list(range(self.nbB))


class Plan:
    pass


def preprocess(edge_index: np.ndarray, cfg: Cfg) -> Plan:
    n, P, nsh, nb = cfg.n, cfg.n_cores, cfg.nsh, cfg.nb
    row = edge_index[0].astype(np.int64)
    col = edge_index[1].astype(np.int64)
    deg = np.bincount(row, minlength=n).astype(np.float64) + 1.0  # + self loop
    dinv = 1.0 / np.sqrt(deg)
    m3 = (-3.0 * dinv).astype(np.float32)           # -3*dinv (dest scale)
    is2 = (-2.0 * deg + 1.0).astype(np.float32)     # identity inject diag

    # source node -> (group, row in half table);  g=0: half A (off>=half_b)
    HB = cfg.half_b
    s = col // nsh
    off = col - s * nsh
    hA_pad = 128 * (nb - cfg.nbB)
    g_of = (off < HB).astype(np.int64)              # g=0 -> half A, g=1 -> B
    rowg = np.where(off >= HB, s * hA_pad + off - HB, s * HB + off)

    ordpos = np.zeros(nb, dtype=np.int64)           # block -> position
    for i, b in enumerate(cfg.ord_blocks):
        ordpos[b] = i
    SGB = cfg.sg_blocks

    core = row // nsh
    per_core_edges = []
    counts = np.zeros((P, nb, 2), dtype=np.int64)
    for r in range(P):
        m = core == r
        rr = row[m] - r * nsh
        cc = rowg[m]
        g = g_of[m]
        blk = rr // 128
        pos = ordpos[blk]
        order = np.lexsort((rr, pos, g, pos // SGB))
        per_core_edges.append((rr[order], cc[order], g[order], blk[order]))
        np.add.at(counts[r], (blk, g), 1)

    nchunks = (counts.max(axis=0) + 127) // 128     # [nb, 2]

    # chunk layout: supergroups of SGB blocks in ord_blocks order; per sg:
    # (g0 chunks for its blocks in order) then (g1 chunks).
    slot_off = {}
    sg_entries = []
    offc = 0
    for i0 in range(0, nb, SGB):
        blocks = cfg.ord_blocks[i0:i0 + SGB]
        entries = []
        for g in (0, 1):
            c0 = offc
            for b in blocks:
                slot_off[(b, g)] = offc
                offc += int(nchunks[b, g]) * 128
            entries.append((g, (offc - c0) // 128, c0 // 128))
        sg_entries.append((blocks, entries))
    total_slots = offc
    total_chunks = total_slots // 128

    per_core = []
    for r in range(P):
        rr, cc, g, blk = per_core_edges[r]
        idx = np.zeros(total_slots, dtype=np.int16)     # pad -> row 0 (valid)
        dstrel = np.full(total_slots, -1.0, dtype=np.float16)
        for b in range(nb):
            for gg in (0, 1):
                msel = (blk == b) & (g == gg)
                k = int(msel.sum())
                if k == 0:
                    continue
                o = slot_off[(b, gg)]
                idx[o:o + k] = cc[msel]
                dstrel[o:o + k] = (rr[msel] - b * 128).astype(np.float16)
        # dma_gather index wrapping: slot i -> [i % 16, i // 16]; the Q7
        # tx/rx cores read different partition groups -> replicate to 128.
        idx16 = np.ascontiguousarray(np.tile(idx.reshape(-1, 16).T, (8, 1)))
        dstrel128 = np.ascontiguousarray(
            dstrel.reshape(total_chunks, 128).T)        # [128, nchunk] fp16

        def colmat(v):
            out = np.zeros((nb * 128,), dtype=np.float64)
            out[:nsh] = v
            return np.ascontiguousarray(
                out.reshape(nb, 128).T.astype(np.float32))

        per_core.append(dict(
            idx16=idx16,
            dstrel=dstrel128,
            dinv_cols=colmat(dinv[r * nsh:(r + 1) * nsh]),
            m3_cols=colmat(m3[r * nsh:(r + 1) * nsh]),
            is2_cols=colmat(is2[r * nsh:(r + 1) * nsh]),
        ))

    plan = Plan()
    plan.cfg = cfg
    plan.nchunks = nchunks
    plan.sg_entries = sg_entries
    plan.total_chunks = total_chunks
    plan.total_slots = total_slots
    plan.per_core = per_core
    return plan


def build_bass(plan: Plan, gates):
    cfg = plan.cfg
    nsh, nb, P = cfg.nsh, cfg.nb, cfg.n_cores
    H, DI, DO, L = cfg.d_h, cfg.d_in, cfg.d_out, cfg.n_layers
    TC = plan.total_chunks
    TS16 = plan.total_slots // 16
    SB = cfg.sgen_batch
    hA = nsh - cfg.half_b
    hA_pad = 128 * (nb - cfg.nbB)
    rowsA, rowsB = P * hA_pad, P * cfg.half_b

    nc = bacc.Bacc("TRN2", target_bir_lowering=False, debug=False,
                   num_devices=P, num_swdge_queues=cfg.n_queues)

    KI = DI // 128
    xT = nc.dram_tensor("xT", [128, KI * nsh], F16, kind="ExternalInput")
    W1 = nc.dram_tensor("W1", [128, KI * H], F16, kind="ExternalInput")
    b1r = nc.dram_tensor("b1r", [1, H], F16, kind="ExternalInput")
    WrT = nc.dram_tensor("WrT", [128, L * H], F16, kind="ExternalInput")
    W2 = nc.dram_tensor("W2", [H, DO], F16, kind="ExternalInput")
    b2r = nc.dram_tensor("b2r", [1, DO], F16, kind="ExternalInput")
    idx16_d = nc.dram_tensor("idx16", [128, TS16], I16, kind="ExternalInput")
    dstrel_d = nc.dram_tensor("dstrel", [128, TC], F16, kind="ExternalInput")
    dinv_d = nc.dram_tensor("dinv_cols", [128, nb], F32, kind="ExternalInput")
    m3_d = nc.dram_tensor("m3_cols", [128, nb], F32, kind="ExternalInput")
    is2_d = nc.dram_tensor("is2_cols", [128, nb], F32, kind="ExternalInput")
    iota_d = nc.dram_tensor("iota_in", [128, SB * 128], F16,
                            kind="ExternalInput")
    ident_d = nc.dram_tensor("ident_in", [128, 128], F16, kind="ExternalInput")
    out_d = nc.dram_tensor("out", [nsh, DO], F32, kind="ExternalOutput")

    last_rows = nsh - (nb - 1) * 128

    def rows_of(b):
        return last_rows if b == nb - 1 else 128

    addr_space = "Shared" if cfg.shared_tables else "Local"

    with tile.TileContext(nc) as tc:
        with (
            tc.tile_pool(name="const", bufs=1) as cpool,
            tc.tile_pool(name="work", bufs=4) as work,
            tc.tile_pool(name="sgen", bufs=4) as sgen_pool,
            tc.tile_pool(name="ga", bufs=cfg.gather_bufs) as ga_pool,
            tc.tile_pool(name="gb", bufs=cfg.gather_bufs) as gb_pool,
            tc.tile_pool(name="pt", bufs=2, space="PSUM") as pt_pool,
            tc.tile_pool(name="ph", bufs=2, space="PSUM") as ph_pool,
            tc.tile_pool(name="pagg", bufs=3, space="PSUM") as pagg_pool,
            tc.tile_pool(name="dram", bufs=2, space="DRAM") as dram,
        ):
            # persistent per-block state (node-major)
            h0 = [cpool.tile([128, 128], F32, tag=f"h0_{b}", name=f"h0_{b}")
                  for b in range(nb)]
            hsb = [cpool.tile([128, 128], F16, tag=f"hs_{b}", name=f"hs_{b}")
                   for b in range(nb)]
            Is2t = [cpool.tile([128, 128], F16, tag=f"i2_{b}", name=f"i2_{b}")
                    for b in range(nb)]
            for b in range(nb):
                nc.vector.memset(h0[b][:], 0.0)

            idx_sb = cpool.tile([128, TS16], I16)
            nc.sync.dma_start(idx_sb[:], idx16_d[:, :])
            dstrel_sb = cpool.tile([128, TC], F16)
            nc.sync.dma_start(dstrel_sb[:], dstrel_d[:, :])
            dinv_sb = cpool.tile([128, nb], F32)
            nc.sync.dma_start(dinv_sb[:], dinv_d[:, :])
            m3_sb = cpool.tile([128, nb], F32)
            nc.sync.dma_start(m3_sb[:], m3_d[:, :])
            is2_sb = cpool.tile([128, nb], F32)
            nc.sync.dma_start(is2_sb[:], is2_d[:, :])

            W1_sb = cpool.tile([128, KI * H], F16)
            nc.sync.dma_start(W1_sb[:], W1[:, :])
            b1_sb = cpool.tile([1, H], F16)
            nc.sync.dma_start(b1_sb[:], b1r[:, :])
            WrT_sb = cpool.tile([128, L * H], F16)
            nc.sync.dma_start(WrT_sb[:], WrT[:, :])
            W2_sb = cpool.tile([H, DO], F16)
            nc.sync.dma_start(W2_sb[:], W2[:, :])
            b2_sb = cpool.tile([1, DO], F16)
            nc.sync.dma_start(b2_sb[:], b2r[:, :])
            ones_sb = cpool.tile([1, 128], F16)
            nc.vector.memset(ones_sb[:], 1.0)
            iota_sb = cpool.tile([128, SB * 128], F16)
            nc.sync.dma_start(iota_sb[:], iota_d[:, :])
            ident = cpool.tile([128, 128], F16)
            nc.sync.dma_start(ident[:], ident_d[:, :])
            for b in range(nb):
                nc.vector.tensor_scalar(Is2t[b][:], ident[:],
                                        is2_sb[:, b:b + 1], None, op0=ALU.mult)

            # ---- lin1: h0 = relu(x @ W1 + b1) (node-major) ----
            for b in range(nb):
                rows = rows_of(b)
                ps = ph_pool.tile([128, 128], F32, tag="ph")
                for k in range(KI):
                    xs = work.tile([128, 128], F16, tag="xs")
                    nc.sync.dma_start(
                        xs[:, :rows],
                        xT[:, k * nsh + b * 128:k * nsh + b * 128 + rows])
                    nc.tensor.matmul(ps[:rows, :], xs[:, :rows],
                                     W1_sb[:, k * H:(k + 1) * H],
                                     start=(k == 0), stop=False)
                nc.tensor.matmul(ps[:rows, :], ones_sb[:, :rows], b1_sb[:],
                                 start=False, stop=True)
                nc.scalar.activation(h0[b][:rows, :], ps[:rows, :], AF.Relu)

            # ---- layers ----
            qrr = [0]

            def next_q():
                q = qrr[0]
                qrr[0] = (q + 1) % cfg.n_queues
                return q

            if cfg.direct_table:
                pidv = nc.sync.partition_id()
                offA_base = nc.sync.compute_val(pidv * (hA_pad * H))
                offB_base = nc.sync.compute_val(pidv * (cfg.half_b * H))

            barr_z = {}
            for l in range(L):
                bounceA = dram.tile([hA_pad, H], F16, tag="bA")
                bounceB = dram.tile([cfg.half_b, H], F16, tag="bB")
                tableA = dram.tile([rowsA, H], F16, tag="tA",
                                   addr_space=addr_space)
                tableB = dram.tile([rowsB, H], F16, tag="tB",
                                   addr_space=addr_space)
                if cfg.direct_table:
                    flagA = dram.tile([1, H], F16, tag="fA")
                    flagB = dram.tile([1, H], F16, tag="fB")
                    flagoutA = dram.tile([P, H], F16, tag="foA")
                    flagoutB = dram.tile([P, H], F16, tag="foB")
                taap = tableA.opt()
                tbap = tableB.opt()
                for b in cfg.ord_blocks:
                    rows = rows_of(b)
                    hb = work.tile([128, 128], F16, tag="hb")
                    nc.scalar.activation(hb[:], h0[b][:], AF.Relu)
                    pt = pt_pool.tile([128, 128], F16, tag="pt")
                    nc.tensor.transpose(pt[:], hb[:], ident[:])
                    hT = work.tile([128, 128], F16, tag="hT")
                    nc.scalar.activation(hT[:], pt[:], AF.Copy)
                    ph = ph_pool.tile([128, 128], F32, tag="ph")
                    nc.tensor.matmul(ph[:], hT[:],
                                     WrT_sb[:, l * H:(l + 1) * H],
                                     start=True, stop=True)
                    nc.scalar.activation(hsb[b][:], ph[:], AF.Copy,
                                         scale=dinv_sb[:, b:b + 1])
                    in_half_a = b >= cfg.nbB
                    r0 = (b - cfg.nbB) * 128 if in_half_a else b * 128
                    bounce = bounceA if in_half_a else bounceB
                    nc.sync.dma_start(bounce[r0:r0 + rows, :],
                                      hsb[b][:rows, :])
                    last = (b == nb - 1) if in_half_a \
                        else (b == cfg.nbB - 1)
                    if last and cfg.direct_table:
                        g = 0 if in_half_a else 1
                        bnc, tap, base, flag, flagout, nrows = (
                            (bounceA, taap, offA_base, flagA, flagoutA, hA)
                            if in_half_a else
                            (bounceB, tbap, offB_base, flagB, flagoutB,
                             cfg.half_b))
                        dst = bass.AP(tap.tensor, base,
                                      [[H, nrows], [1, H]],
                                      dep_tracking_offset=0)
                        nc.sync.dma_start(dst, bnc.opt())
                        fsb = work.tile([1, H], F16, tag="flag")
                        nc.sync.dma_start(fsb[:], tap[0:1, :])
                        nc.sync.dma_start(flag[0:1, :], fsb[:])
                        nc.gpsimd.collective_compute(
                            "AllGather", ALU.bypass,
                            replica_groups=[list(range(P))],
                            ins=[flag.opt()], outs=[flagout.opt()])
                        bsb = work.tile([1, 16], F16, tag="barr",
                                        name=f"bsb{g}_{l}")
                        nc.sync.dma_start(bsb[:], flagout[0:1, 0:16])
                        breg = nc.gpsimd.alloc_register(f"barr_{g}_{l}")
                        nc.gpsimd.reg_load(
                            breg, bsb.bitcast(mybir.dt.uint32)[0:1, 0:1])
                        barr_z[g] = nc.gpsimd.compute_val(
                            nc.gpsimd.snap(breg, donate=True) * 0)
                    elif last:
                        bnc, tbl_t = ((bounceA, tableA) if in_half_a
                                      else (bounceB, tableB))
                        nc.gpsimd.collective_compute(
                            "AllGather", ALU.bypass,
                            replica_groups=[list(range(P))],
                            ins=[bnc.opt()], outs=[tbl_t.opt()])

                tbl = {0: tableA.opt(), 1: tableB.opt()}
                gp = {0: ga_pool, 1: gb_pool}
                for si, (blocks, entries) in enumerate(plan.sg_entries):
                    bufs = {}
                    base_chunk = {}
                    for (g, nch, ch0) in entries:
                        if nch == 0:
                            continue
                        mb = gp[g].tile([128, nch, H], F16, tag=f"g{g}",
                                        name=f"mb{g}_{si}")
                        for t0 in range(0, nch, cfg.cmax):
                            tn = min(cfg.cmax, nch - t0)
                            c0 = ch0 + t0
                            nreg = (barr_z[g] + tn * 128) \
                                if cfg.direct_table else tn * 128
                            nc.gpsimd.dma_gather(
                                mb[:, t0:t0 + tn, :], tbl[g],
                                idx_sb[:, c0 * 8:(c0 + tn) * 8],
                                num_idxs=tn * 128, num_idxs_reg=nreg,
                                elem_size=H, queue_num=next_q(),
                                single_packet=False)
                        bufs[g] = mb
                        base_chunk[g] = ch0
                    for b in blocks:
                        nch_tot = int(plan.nchunks[b, 0] + plan.nchunks[b, 1])
                        bi = blocks.index(b)
                        pa = pagg_pool.tile([128, 128], F32, tag="pagg")
                        nc.tensor.matmul(pa[:], Is2t[b][:], hsb[b][:],
                                         start=True, stop=(nch_tot == 0))
                        done = 0
                        for g in (0, 1):
                            nch_bg = int(plan.nchunks[b, g])
                            if nch_bg == 0:
                                continue
                            loc = sum(int(plan.nchunks[bb, g])
                                      for bb in blocks[:bi])
                            mb = bufs[g]
                            for t0 in range(0, nch_bg, SB):
                                tn = min(SB, nch_bg - t0)
                                c = base_chunk[g] + loc + t0
                                sg_t = sgen_pool.tile([128, SB * 128], F16,
                                                      tag="sg")
                                dsl = dstrel_sb[:, c:c + tn]
                                nc.vector.tensor_tensor(
                                    sg_t[:, :tn * 128].rearrange(
                                        "p (c d) -> p c d", d=128),
                                    dsl.unsqueeze(2).broadcast_to(
                                        [128, tn, 128]),
                                    iota_sb[:, :tn * 128].rearrange(
                                        "p (c d) -> p c d", d=128),
                                    op=ALU.is_equal)
                                for t in range(tn):
                                    done += 1
                                    nc.tensor.matmul(
                                        pa[:],
                                        sg_t[:, t * 128:(t + 1) * 128],
                                        mb[:, loc + t0 + t, :],
                                        start=False, stop=(done == nch_tot))
                        hm = work.tile([128, 128], F32, tag="hm")
                        nc.scalar.activation(hm[:], pa[:], AF.Copy,
                                             scale=m3_sb[:, b:b + 1])
                        nc.vector.scalar_tensor_tensor(
                            h0[b][:], h0[b][:], gates[l], hm[:],
                            op0=ALU.mult, op1=ALU.add)

            # ---- lin2 ----
            for b in range(nb):
                rows = rows_of(b)
                hb = work.tile([128, 128], F16, tag="hb")
                nc.scalar.activation(hb[:], h0[b][:], AF.Relu)
                pt = pt_pool.tile([128, 128], F16, tag="pt")
                nc.tensor.transpose(pt[:], hb[:], ident[:])
                hT = work.tile([128, 128], F16, tag="hT")
                nc.scalar.activation(hT[:], pt[:], AF.Copy)
                po = ph_pool.tile([128, DO], F32, tag="ph", name="po")
                nc.tensor.matmul(po[:rows, :], hT[:, :rows], W2_sb[:, :],
                                 start=True, stop=False)
                nc.tensor.matmul(po[:rows, :], ones_sb[:, :rows], b2_sb[:],
                                 start=False, stop=True)
                ot = work.tile([128, DO], F32, tag="ot")
                nc.scalar.activation(ot[:rows, :], po[:rows, :], AF.Copy)
                nc.sync.dma_start(out_d[b * 128:b * 128 + rows, :],
                                  ot[:rows, :])

    nc.finalize()
    return nc


def make_in_maps(plan: Plan, x, W1, b1, Wr, W2, b2):
    cfg = plan.cfg
    nsh = cfg.nsh
    KI = cfg.d_in // 128
    W1m = np.ascontiguousarray(
        np.asarray(W1, np.float32).reshape(KI, 128, cfg.d_h)
        .transpose(1, 0, 2).reshape(128, KI * cfg.d_h)).astype(np.float16)
    # WrT[k, l*H+j] = Wr[l, j, k]
    WrTm = np.ascontiguousarray(
        np.asarray(Wr, np.float32).transpose(2, 0, 1)
        .reshape(128, -1)).astype(np.float16)
    iota_in = np.ascontiguousarray(
        np.tile(np.arange(128, dtype=np.float16)[None, :],
                (128, cfg.sgen_batch)))
    ident_in = np.eye(128, dtype=np.float16)
    common = dict(
        W1=W1m,
        b1r=np.ascontiguousarray(
            np.asarray(b1, np.float32).reshape(1, -1)).astype(np.float16),
        WrT=WrTm,
        W2=np.ascontiguousarray(np.asarray(W2, np.float32)).astype(np.float16),
        b2r=np.ascontiguousarray(
            np.asarray(b2, np.float32).reshape(1, -1)).astype(np.float16),
        iota_in=iota_in, ident_in=ident_in,
    )
    in_maps = []
    x = np.asarray(x, np.float32)
    for r in range(cfg.n_cores):
        pc = plan.per_core[r]
        # xT[p, k*nsh + c] = x[c, k*128 + p]
        xTm = np.ascontiguousarray(
            x[r * nsh:(r + 1) * nsh].T
            .reshape(KI, 128, nsh).transpose(1, 0, 2)
            .reshape(128, KI * nsh)).astype(np.float16)
        m = dict(common)
        m.update(
            xT=xTm, idx16=pc["idx16"], dstrel=pc["dstrel"],
            dinv_cols=pc["dinv_cols"], m3_cols=pc["m3_cols"],
            is2_cols=pc["is2_cols"],
        )
        in_maps.append(m)
    return in_maps


_cache = {}


def kernel(x, W1, b1, Wr, eps, W2, b2, edge_index, *, trace=False, cfg=None):
    cfg = cfg or Cfg()
    x = np.asarray(x)
    edge_index = np.asarray(edge_index)
    gates = [float(1.0 + math.tanh(float(e))) for e in np.asarray(eps)]

    ck = hash((edge_index.tobytes(), tuple(gates), cfg.n, cfg.n_cores,
               cfg.sg_blocks, cfg.n_queues, cfg.cmax, cfg.shared_tables,
               cfg.direct_table))
    if ck not in _cache:
        plan = preprocess(edge_index, cfg)
        nc = build_bass(plan, gates)
        _cache.clear()
        _cache[ck] = (plan, nc)
    plan, nc = _cache[ck]

    in_maps = make_in_maps(plan, x, W1, b1, Wr, W2, b2)
    try:
        res = bass_utils.run_bass_kernel_spmd(
            nc, in_maps, core_ids=list(range(cfg.n_cores)), trace=trace)
    except ModuleNotFoundError:
        # axon NTFF profiling hook unavailable in this container
        res = bass_utils.run_bass_kernel_spmd(
            nc, in_maps, core_ids=list(range(cfg.n_cores)), trace=False)
    out = np.concatenate([r["out"] for r in res.results], axis=0)
    kernel.last_results = res
    return out.astype(np.float32)
